# revision 1
# baseline (speedup 1.0000x reference)
"""nn_MoEMLP — Trainium2 Bass kernel (8 NeuronCores, expert-parallel).

kernel(**inputs) takes the FULL unsharded inputs (as produced by
setup_inputs) and returns the FULL output [4, 2048, 1024] fp32.

Strategy (per core i == expert i, one SPMD program):
  - fp32 router on the core's 1024-token shard (logits -> softmax -> top-2)
  - AllGather of the per-shard top-2 masks -> global mask [8192, 8]
  - global positions via per-row prefix scans + triangular matmuls;
    capacity masking (C=1280); a fully static slot->token inversion built
    from one-hot selection against kept-row prefixes (no data-dependent
    scatter)
  - dispatch: 10 indirect row-gathers of x, PE-transpose to bufT [1024, 1280]
  - expert FFN in bf16: hT = gelu(w1.T @ bufT + b1); out = hT.T @ w2 + b2
  - AllGather of expert outputs (bf16) -> [10240, 1024]
  - combine: per-token weighted sum of its two expert rows via indirect
    gathers; each core emits its token shard of y; host concatenates.
"""
import numpy as np
from contextlib import ExitStack

import concourse.bass as bass
import concourse.mybir as mybir
import concourse.tile as tile
from concourse import bacc, bass_utils

F32 = mybir.dt.float32
BF16 = mybir.dt.bfloat16
I32 = mybir.dt.int32
U32 = mybir.dt.uint32
AF = mybir.ActivationFunctionType
OP = mybir.AluOpType

P = 128
D = 1024
DH = 4096
E = 8
NCORE = 8
NTOK = 8192
TSH = 1024
CAP = 1280
JW = 64
CB = 256
NCB = 5
SC = 10

_CACHE = {}


def _build():
    nc = bacc.Bacc("TRN2", target_bir_lowering=False, debug=False, num_devices=NCORE)

    x = nc.dram_tensor("x", [NTOK, D], F32, kind="ExternalInput")
    xs = nc.dram_tensor("xs", [TSH, D], F32, kind="ExternalInput")
    rw = nc.dram_tensor("rw", [D, E], F32, kind="ExternalInput")
    rb = nc.dram_tensor("rb", [1, E], F32, kind="ExternalInput")
    w1 = nc.dram_tensor("w1", [D, DH], BF16, kind="ExternalInput")
    w2 = nc.dram_tensor("w2", [DH, D], BF16, kind="ExternalInput")
    b1 = nc.dram_tensor("b1", [1, DH], F32, kind="ExternalInput")
    b2 = nc.dram_tensor("b2", [1, D], F32, kind="ExternalInput")
    ohcol = nc.dram_tensor("ohcol", [P, E], F32, kind="ExternalInput")
    sel16 = nc.dram_tensor("sel16", [P, 1], F32, kind="ExternalInput")
    y = nc.dram_tensor("y", [TSH, D], F32, kind="ExternalOutput")

    agin1 = nc.dram_tensor("agin1", [TSH, E], F32, kind="Internal")
    gmask = nc.dram_tensor("gmask", [NTOK, E], F32, kind="Internal", addr_space="Shared")
    agin2 = nc.dram_tensor("agin2", [CAP, D], BF16, kind="Internal")
    outall = nc.dram_tensor("outall", [E * CAP, D], BF16, kind="Internal", addr_space="Shared")

    ident_f = nc.inline_tensor(np.eye(P, dtype=np.float32), "ident_f")
    tri_np = (np.arange(P)[:, None] < np.arange(P)[None, :]).astype(np.float32)
    tril = nc.inline_tensor(tri_np, "tril")
    ones_1x128 = nc.inline_tensor(np.ones((1, P), np.float32), "ones_1x128")
    ones_128x1 = nc.inline_tensor(np.ones((P, 1), np.float32), "ones_128x1")
    ones_64x1 = nc.inline_tensor(np.ones((64, 1), np.float32), "ones_64x1")
    ones_1x64 = nc.inline_tensor(np.ones((1, 64), np.float32), "ones_1x64")
    iota_sf = nc.inline_tensor(
        np.broadcast_to(np.arange(P, dtype=np.float32), (P, P)).copy(), "iota_sf")
    iota_rp = nc.inline_tensor(
        np.broadcast_to(np.arange(64, dtype=np.float32)[:, None], (64, P)).copy(), "iota_rp")
    iota_row = nc.inline_tensor(np.arange(P, dtype=np.float32)[None, :].copy(), "iota_row")
    keysraw = nc.inline_tensor(
        np.broadcast_to((JW - np.arange(JW)).astype(np.float32), (P, JW)).copy(), "keysraw")
    iota_e1280 = nc.inline_tensor(
        np.broadcast_to((np.arange(E) * CAP).astype(np.float32), (P, E)).copy(), "iota_e1280")
    pval = nc.inline_tensor(np.arange(P, dtype=np.float32)[:, None].copy(), "pval")

    with tile.TileContext(nc) as tc, ExitStack() as ctx:
        pp = ctx.enter_context(tc.tile_pool(name="persist", bufs=1))
        wk = ctx.enter_context(tc.tile_pool(name="work", bufs=2))
        psT = ctx.enter_context(tc.tile_pool(name="psT", bufs=2, space="PSUM"))
        psS = ctx.enter_context(tc.tile_pool(name="psS", bufs=2, space="PSUM"))
        ps1p = ctx.enter_context(tc.tile_pool(name="ps1p", bufs=2, space="PSUM"))
        ps2p = ctx.enter_context(tc.tile_pool(name="ps2p", bufs=1, space="PSUM"))

        def t(pool, shape, dt, tag):
            return pool.tile(shape, dt, tag=tag, name=tag)

        idf = t(pp, [P, P], F32, "idf"); nc.sync.dma_start(idf[:], ident_f.ap())
        trl = t(pp, [P, P], F32, "trl"); nc.sync.dma_start(trl[:], tril.ap())
        o1x128 = t(pp, [1, P], F32, "o1x128"); nc.sync.dma_start(o1x128[:], ones_1x128.ap())
        o128x1 = t(pp, [P, 1], F32, "o128x1"); nc.sync.dma_start(o128x1[:], ones_128x1.ap())
        o64x1 = t(pp, [64, 1], F32, "o64x1"); nc.sync.dma_start(o64x1[:], ones_64x1.ap())
        o1x64 = t(pp, [1, 64], F32, "o1x64"); nc.sync.dma_start(o1x64[:], ones_1x64.ap())
        isf = t(pp, [P, P], F32, "isf"); nc.sync.dma_start(isf[:], iota_sf.ap())
        irp = t(pp, [64, P], F32, "irp"); nc.sync.dma_start(irp[:], iota_rp.ap())
        irow = t(pp, [1, P], F32, "irow"); nc.sync.dma_start(irow[:], iota_row.ap())
        kraw = t(pp, [P, JW], F32, "kraw"); nc.sync.dma_start(kraw[:], keysraw.ap())
        ie1280 = t(pp, [P, E], F32, "ie1280"); nc.sync.dma_start(ie1280[:], iota_e1280.ap())
        pvl = t(pp, [P, 1], F32, "pvl"); nc.sync.dma_start(pvl[:], pval.ap())
        ohc = t(pp, [P, E], F32, "ohc"); nc.sync.dma_start(ohc[:], ohcol.ap())
        s16 = t(pp, [P, 1], F32, "s16"); nc.sync.dma_start(s16[:], sel16.ap())
        rb_sb = t(pp, [1, E], F32, "rb_sb"); nc.sync.dma_start(rb_sb[:], rb.ap())
        rw_sb = t(pp, [P, 8 * E], F32, "rw_sb")
        for k in range(8):
            nc.sync.dma_start(rw_sb[:, k * E : (k + 1) * E], rw.ap()[k * P : (k + 1) * P, :])
        b2_sb = t(pp, [1, D], F32, "b2_sb"); nc.sync.dma_start(b2_sb[:], b2.ap())
        b1r = t(wk, [32, P], F32, "b1r")
        nc.sync.dma_start(b1r[:], b1.ap().rearrange("o (m p) -> (o m) p", p=P))
        b1p = psT.tile([P, 32], F32, tag="tp")
        nc.tensor.transpose(out=b1p[:], in_=b1r[:], identity=idf[:32, :32])
        b1t = t(pp, [P, 32], F32, "b1t")
        nc.vector.tensor_copy(b1t[:], b1p[:])

        gates = [t(pp, [P, E], F32, f"gates{c}") for c in range(8)]
        lmask = [t(pp, [P, E], F32, f"lmask{c}") for c in range(8)]
        repl1 = [t(pp, [P, E], F32, f"repl1{c}") for c in range(8)]
        repl2 = [t(pp, [P, E], F32, f"repl2{c}") for c in range(8)]
        rpr = t(pp, [P, E], F32, "rpr")
        rkp = t(pp, [P, 1], F32, "rkp")
        rkpn = t(pp, [P, 1], F32, "rkpn")
        idxc = [t(pp, [P, 1], I32, f"idxc{S}") for S in range(SC)]

        # ---------------- Phase R: router ----------------
        with tc.tile_pool(name="rpool", bufs=1) as rp:
            xT = [rp.tile([P, TSH], F32, tag=f"xT{k}", name=f"xT{k}") for k in range(8)]
            for c in range(8):
                xsc = t(wk, [P, D], F32, "xsc")
                nc.sync.dma_start(xsc[:], xs.ap()[c * P : (c + 1) * P, :])
                for k in range(8):
                    tp = psT.tile([P, P], F32, tag="tp")
                    nc.tensor.transpose(out=tp[:], in_=xsc[:, k * P : (k + 1) * P], identity=idf[:])
                    nc.vector.tensor_copy(xT[k][:, c * P : (c + 1) * P], tp[:])
            for c in range(8):
                lg = psS.tile([P, E], F32, tag="ss")
                nc.tensor.matmul(lg[:], lhsT=o1x128[:], rhs=rb_sb[:], start=True, stop=False)
                for k in range(8):
                    nc.tensor.matmul(
                        lg[:], lhsT=xT[k][:, c * P : (c + 1) * P],
                        rhs=rw_sb[:, k * E : (k + 1) * E], start=False, stop=(k == 7))
                l_sb = t(wk, [P, E], F32, "l_sb")
                nc.vector.tensor_copy(l_sb[:], lg[:])
                m1 = t(wk, [P, 1], F32, "m1")
                nc.vector.reduce_max(m1[:], l_sb[:], axis=mybir.AxisListType.X)
                nm = t(wk, [P, 1], F32, "nm")
                nc.vector.tensor_scalar_mul(nm[:], m1[:], -1.0)
                ex = t(wk, [P, E], F32, "ex")
                ssum = t(wk, [P, 1], F32, "ssum")
                nc.scalar.activation(ex[:], l_sb[:], AF.Exp, bias=nm[:, :1], accum_out=ssum[:, :1])
                rcp = t(wk, [P, 1], F32, "rcp")
                nc.vector.reciprocal(rcp[:], ssum[:])
                nc.vector.tensor_scalar_mul(gates[c][:], ex[:], rcp[:, :1])
                mx = t(wk, [P, E], F32, "mx")
                nc.vector.max(out=mx[:], in_=gates[c][:])
                t1r = t(wk, [P, E], F32, "t1r")
                nc.vector.tensor_copy(t1r[:], mx[:])
                nc.vector.memset(t1r[:, 1:E], -2.0)
                nc.vector.match_replace(out=repl1[c][:], in_to_replace=t1r[:], in_values=gates[c][:], imm_value=-3.0)
                t2r = t(wk, [P, E], F32, "t2r")
                nc.vector.tensor_copy(t2r[:, 0:1], mx[:, 1:2])
                nc.vector.memset(t2r[:, 1:E], -2.0)
                nc.vector.match_replace(out=repl2[c][:], in_to_replace=t2r[:], in_values=repl1[c][:], imm_value=-3.0)
                nc.vector.tensor_tensor(out=lmask[c][:], in0=gates[c][:], in1=repl2[c][:], op=OP.is_gt)
                nc.sync.dma_start(agin1.ap()[c * P : (c + 1) * P, :], lmask[c][:])

        nc.gpsimd.collective_compute(
            "AllGather", OP.bypass, replica_groups=[list(range(NCORE))],
            ins=[agin1.ap()], outs=[gmask.ap()])

        # ---------------- Phase P + D ----------------
        bufT = [t(pp, [P, CAP], BF16, f"bufT{k}") for k in range(8)]
        with tc.tile_pool(name="ppool", bufs=1) as pq:
            gm = pq.tile([P, 512], F32, tag="gm", name="gm")
            nc.sync.dma_start(gm[:], gmask.ap().rearrange("(p j) e -> p (j e)", p=P))
            z512 = pq.tile([P, 512], F32, tag="z512", name="z512")
            nc.vector.memset(z512[:], 0.0)
            sc_all = pq.tile([P, 512], F32, tag="sc_all", name="sc_all")
            for e in range(E):
                nc.vector.tensor_tensor_scan(
                    out=sc_all[:, e::E], data0=gm[:, e::E], data1=z512[:, e::E],
                    initial=-1.0, op0=OP.add, op1=OP.add)
            rtot = t(wk, [P, E], F32, "rtot")
            nc.vector.tensor_scalar_add(rtot[:], sc_all[:, 504:512], 1.0)
            rprp = psS.tile([P, E], F32, tag="ss")
            nc.tensor.matmul(rprp[:], lhsT=trl[:], rhs=rtot[:], start=True, stop=True)
            nc.vector.tensor_copy(rpr[:], rprp[:])
            for e in range(E):
                nc.vector.tensor_scalar_add(sc_all[:, e::E], sc_all[:, e::E], rpr[:, e : e + 1])
            pen = pq.tile([P, 512], F32, tag="pen", name="pen")
            nc.vector.tensor_scalar(
                out=pen[:], in0=gm[:], scalar1=-1.0e6, scalar2=1.0e6, op0=OP.mult, op1=OP.add)
            pdm = pq.tile([P, 512], F32, tag="pdm", name="pdm")
            nc.vector.tensor_add(pdm[:], sc_all[:], pen[:])
            pd_e = pq.tile([P, JW], F32, tag="pd_e", name="pd_e")
            tsel = t(wk, [P, JW], F32, "tsel")
            nc.vector.tensor_scalar_mul(pd_e[:], pdm[:, 0::E], ohc[:, 0:1])
            for k in range(1, E):
                nc.vector.tensor_scalar_mul(tsel[:], pdm[:, k::E], ohc[:, k : k + 1])
                nc.vector.tensor_add(pd_e[:], pd_e[:], tsel[:])
            kept = pq.tile([P, JW], F32, tag="kept", name="kept")
            nc.vector.tensor_scalar(
                out=kept[:], in0=pd_e[:], scalar1=float(CAP) - 0.5, scalar2=None, op0=OP.is_le)
            keyA = pq.tile([P, JW], F32, tag="keyA", name="keyA")
            keyB = pq.tile([P, JW], F32, tag="keyB", name="keyB")
            nc.vector.tensor_mul(keyA[:], kraw[:], kept[:])
            ktmp = t(wk, [P, JW], F32, "ktmp")
            nc.vector.tensor_scalar_add(ktmp[:], kept[:], -1.0)
            nc.vector.tensor_add(keyA[:], keyA[:], ktmp[:])
            loc = pq.tile([P, JW], F32, tag="loc", name="loc")
            cur, nxt = keyA, keyB
            for r8 in range(8):
                mx8 = t(wk, [P, 8], F32, "mx8")
                nc.vector.max(out=mx8[:], in_=cur[:])
                mi8 = t(wk, [P, 8], U32, "mi8")
                nc.vector.max_index(out=mi8[:], in_max=mx8[:], in_values=cur[:])
                nc.vector.tensor_copy(loc[:, r8 * 8 : (r8 + 1) * 8], mi8[:])
                if r8 < 7:
                    nc.vector.match_replace(out=nxt[:], in_to_replace=mx8[:], in_values=cur[:], imm_value=-1.0)
                    cur, nxt = nxt, cur
            rcnt = t(wk, [P, 1], F32, "rcnt")
            nc.vector.reduce_sum(rcnt[:], kept[:], axis=mybir.AxisListType.X)
            rkpp = psS.tile([P, 1], F32, tag="ss")
            nc.tensor.matmul(rkpp[:], lhsT=trl[:], rhs=rcnt[:], start=True, stop=True)
            nc.vector.tensor_copy(rkp[:], rkpp[:])
            nc.vector.tensor_add(rkpn[:], rkp[:], rcnt[:])

            for S in range(SC):
                rpS = t(wk, [P, 1], F32, "rpS")
                nc.vector.tensor_scalar_add(rpS[:], rkp[:], float(-P * S))
                rpnS = t(wk, [P, 1], F32, "rpnS")
                nc.vector.tensor_scalar_add(rpnS[:], rkpn[:], float(-P * S))
                selA = t(wk, [P, P], F32, "selA")
                nc.vector.tensor_scalar(out=selA[:], in0=isf[:], scalar1=rpS[:, :1], scalar2=None, op0=OP.is_ge)
                selB = t(wk, [P, P], F32, "selB")
                nc.vector.tensor_scalar(out=selB[:], in0=isf[:], scalar1=rpnS[:, :1], scalar2=None, op0=OP.is_lt)
                selO = t(wk, [P, P], F32, "selO")
                nc.vector.tensor_mul(selO[:], selA[:], selB[:])
                rap = psS.tile([1, P], F32, tag="ss")
                nc.tensor.matmul(rap[:], lhsT=rkp[:], rhs=selO[:], start=True, stop=True)
                psp = psS.tile([1, P], F32, tag="ss")
                nc.tensor.matmul(psp[:], lhsT=pvl[:], rhs=selO[:], start=True, stop=True)
                pst = t(wk, [1, P], F32, "pst")
                nc.vector.tensor_copy(pst[:], psp[:])
                rsr = t(wk, [1, P], F32, "rsr")
                nc.vector.tensor_scalar_add(rsr[:], irow[:], float(P * S))
                nc.vector.tensor_sub(rsr[:], rsr[:], rap[:])
                Tp = psS.tile([64, P], F32, tag="ss")
                nc.tensor.matmul(Tp[:], lhsT=loc[:], rhs=selO[:], start=True, stop=True)
                repp = psS.tile([64, P], F32, tag="ss")
                nc.tensor.matmul(repp[:], lhsT=o1x64[:], rhs=rsr[:], start=True, stop=True)
                Rm = t(wk, [64, P], F32, "Rm")
                nc.vector.tensor_tensor(out=Rm[:], in0=irp[:], in1=repp[:], op=OP.is_equal)
                RT = t(wk, [64, P], F32, "RT")
                nc.vector.tensor_mul(RT[:], Rm[:], Tp[:])
                srow = psS.tile([1, P], F32, tag="ss")
                nc.tensor.matmul(srow[:], lhsT=o64x1[:], rhs=RT[:], start=True, stop=True)
                idxr = t(wk, [1, P], F32, "idxr")
                nc.vector.tensor_scalar_mul(idxr[:], pst[:], float(JW))
                nc.vector.tensor_add(idxr[:], idxr[:], srow[:])
                itp = psS.tile([P, 1], F32, tag="ss")
                nc.tensor.transpose(out=itp[:], in_=idxr[:], identity=idf[:1, :1])
                nc.vector.tensor_copy(idxc[S][:], itp[:])

            for S in range(SC):
                xg = t(wk, [P, D], F32, "xg")
                nc.gpsimd.indirect_dma_start(
                    out=xg[:], out_offset=None, in_=x.ap(),
                    in_offset=bass.IndirectOffsetOnAxis(ap=idxc[S][:, :1], axis=0))
                for k in range(8):
                    tp = psT.tile([P, P], F32, tag="tp")
                    nc.tensor.transpose(out=tp[:], in_=xg[:, k * P : (k + 1) * P], identity=idf[:])
                    nc.vector.tensor_copy(bufT[k][:, S * P : (S + 1) * P], tp[:])

        # ---------------- Phase G: expert FFN ----------------
        with tc.tile_pool(name="fpool", bufs=1) as fp:
            w1sb = [fp.tile([P, DH], BF16, tag=f"w1sb{k}", name=f"w1sb{k}") for k in range(8)]
            for k in range(8):
                nc.sync.dma_start(w1sb[k][:], w1.ap()[k * P : (k + 1) * P, :])
            hT = fp.tile([P, 32 * CB], BF16, tag="hT", name="hT")
            for cb in range(NCB):
                c0 = cb * CB
                for m in range(32):
                    ps1 = ps1p.tile([P, CB], F32, tag="ps1")
                    for k in range(8):
                        nc.tensor.matmul(
                            ps1[:], lhsT=w1sb[k][:, m * P : (m + 1) * P],
                            rhs=bufT[k][:, c0 : c0 + CB], start=(k == 0), stop=(k == 7))
                    nc.scalar.activation(
                        hT[:, m * CB : (m + 1) * CB], ps1[:], AF.Gelu, bias=b1t[:, m : m + 1])
                orow = t(wk, [P, D], BF16, "orow")
                orow2 = t(wk, [P, D], BF16, "orow2")
                ot = (orow, orow2)
                for dn in range(2):
                    ps2a = ps2p.tile([P, 512], F32, tag="ps2a")
                    ps2b = ps2p.tile([P, 512], F32, tag="ps2b")
                    pt2 = (ps2a, ps2b)
                    for cc in range(2):
                        nc.tensor.matmul(
                            pt2[cc][:], lhsT=o1x128[:], rhs=b2_sb[:, dn * 512 : (dn + 1) * 512],
                            start=True, stop=False)
                    for h in range(32):
                        w2h = fp.tile([P, 512], BF16, tag="w2h", name="w2h", bufs=3)
                        nc.sync.dma_start(w2h[:], w2.ap()[h * P : (h + 1) * P, dn * 512 : (dn + 1) * 512])
                        for cc in range(2):
                            nc.tensor.matmul(
                                pt2[cc][:], lhsT=hT[:, h * CB + cc * P : h * CB + (cc + 1) * P],
                                rhs=w2h[:], start=False, stop=(h == 31))
                    for cc in range(2):
                        nc.vector.tensor_copy(ot[cc][:, dn * 512 : (dn + 1) * 512], pt2[cc][:])
                for cc in range(2):
                    nc.sync.dma_start(agin2.ap()[c0 + cc * P : c0 + (cc + 1) * P, :], ot[cc][:])

        nc.gpsimd.collective_compute(
            "AllGather", OP.bypass, replica_groups=[list(range(NCORE))],
            ins=[agin2.ap()], outs=[outall.ap()])

        # ---------------- Phase C: combine ----------------
        basep = psS.tile([1, E], F32, tag="ss")
        nc.tensor.matmul(basep[:], lhsT=s16[:], rhs=rpr[:], start=True, stop=True)
        cum = t(wk, [1, E], F32, "cum0")
        nc.vector.tensor_copy(cum[:], basep[:])
        for c in range(8):
            lpp = psS.tile([P, E], F32, tag="ss")
            nc.tensor.matmul(lpp[:], lhsT=o1x128[:], rhs=cum[:], start=True, stop=False)
            nc.tensor.matmul(lpp[:], lhsT=trl[:], rhs=lmask[c][:], start=False, stop=True)
            lp = t(wk, [P, E], F32, "lp")
            nc.vector.tensor_copy(lp[:], lpp[:])
            if c < 7:
                totp = psS.tile([1, E], F32, tag="ss")
                nc.tensor.matmul(totp[:], lhsT=o128x1[:], rhs=lmask[c][:], start=True, stop=True)
                ncum = t(wk, [1, E], F32, "cumN")
                nc.vector.tensor_add(ncum[:], cum[:], totp[:])
                cum = ncum
            va = t(wk, [P, E], F32, "va")
            nc.vector.tensor_scalar(out=va[:], in0=lp[:], scalar1=float(CAP) - 0.5, scalar2=None, op0=OP.is_le)
            lval = t(wk, [P, E], F32, "lval")
            nc.vector.tensor_mul(lval[:], va[:], lmask[c][:])
            lpc = t(wk, [P, E], F32, "lpc")
            nc.vector.tensor_scalar_min(lpc[:], lp[:], float(CAP - 1))
            rowid = t(wk, [P, E], F32, "rowid")
            nc.vector.tensor_add(rowid[:], lpc[:], ie1280[:])
            gv = t(wk, [P, E], F32, "gv")
            nc.vector.tensor_mul(gv[:], gates[c][:], lval[:])
            is1 = t(wk, [P, E], F32, "is1")
            nc.vector.tensor_tensor(out=is1[:], in0=gates[c][:], in1=repl1[c][:], op=OP.is_gt)
            is2 = t(wk, [P, E], F32, "is2")
            nc.vector.tensor_tensor(out=is2[:], in0=repl1[c][:], in1=repl2[c][:], op=OP.is_gt)
            junk = t(wk, [P, E], F32, "junk")
            w1s = t(wk, [P, 1], F32, "w1s")
            nc.vector.tensor_mul(junk[:], is1[:], gv[:])
            nc.vector.reduce_sum(w1s[:], junk[:], axis=mybir.AxisListType.X)
            w2s = t(wk, [P, 1], F32, "w2s")
            nc.vector.tensor_mul(junk[:], is2[:], gv[:])
            nc.vector.reduce_sum(w2s[:], junk[:], axis=mybir.AxisListType.X)
            g1f = t(wk, [P, 1], F32, "g1f")
            nc.vector.tensor_mul(junk[:], is1[:], rowid[:])
            nc.vector.reduce_sum(g1f[:], junk[:], axis=mybir.AxisListType.X)
            g2f = t(wk, [P, 1], F32, "g2f")
            nc.vector.tensor_mul(junk[:], is2[:], rowid[:])
            nc.vector.reduce_sum(g2f[:], junk[:], axis=mybir.AxisListType.X)
            g1i = t(wk, [P, 1], I32, "g1i")
            nc.vector.tensor_copy(g1i[:], g1f[:])
            g2i = t(wk, [P, 1], I32, "g2i")
            nc.vector.tensor_copy(g2i[:], g2f[:])
            r1 = t(wk, [P, D], BF16, "r1")
            nc.gpsimd.indirect_dma_start(
                out=r1[:], out_offset=None, in_=outall.ap(),
                in_offset=bass.IndirectOffsetOnAxis(ap=g1i[:, :1], axis=0))
            r2 = t(wk, [P, D], BF16, "r2")
            nc.gpsimd.indirect_dma_start(
                out=r2[:], out_offset=None, in_=outall.ap(),
                in_offset=bass.IndirectOffsetOnAxis(ap=g2i[:, :1], axis=0))
            y1 = t(wk, [P, D], F32, "y1")
            nc.vector.tensor_scalar_mul(y1[:], r1[:], w1s[:, :1])
            y2 = t(wk, [P, D], F32, "y2")
            nc.scalar.activation(y2[:], r2[:], AF.Copy, scale=w2s[:, :1])
            yc = t(wk, [P, D], F32, "yc")
            nc.vector.tensor_add(yc[:], y1[:], y2[:])
            nc.sync.dma_start(y.ap()[c * P : (c + 1) * P, :], yc[:])

    nc.compile()
    return nc


def _make_in_maps(inputs):
    import ml_dtypes

    x = np.ascontiguousarray(np.asarray(inputs["x"], np.float32).reshape(NTOK, D))
    rw = np.ascontiguousarray(np.asarray(inputs["router_w"], np.float32))
    rb = np.ascontiguousarray(np.asarray(inputs["router_b"], np.float32)).reshape(1, E)
    w1 = np.asarray(inputs["w1"])
    w2 = np.asarray(inputs["w2"])
    b1 = np.asarray(inputs["b1"])
    b2 = np.asarray(inputs["b2"])
    in_maps = []
    for i in range(NCORE):
        oh = np.zeros((P, E), np.float32)
        oh[:, i] = 1.0
        s16 = np.zeros((P, 1), np.float32)
        s16[16 * i, 0] = 1.0
        in_maps.append({
            "x": x,
            "xs": np.ascontiguousarray(x[i * TSH : (i + 1) * TSH]),
            "rw": rw,
            "rb": rb,
            "w1": np.ascontiguousarray(np.asarray(w1[i], np.float32).astype(ml_dtypes.bfloat16)),
            "w2": np.ascontiguousarray(np.asarray(w2[i], np.float32).astype(ml_dtypes.bfloat16)),
            "b1": np.ascontiguousarray(np.asarray(b1[i], np.float32)).reshape(1, DH),
            "b2": np.ascontiguousarray(np.asarray(b2[i], np.float32)).reshape(1, D),
            "ohcol": oh,
            "sel16": s16,
        })
    return in_maps


def run(inputs, trace=False):
    if "nc" not in _CACHE:
        _CACHE["nc"] = _build()
    nc = _CACHE["nc"]
    in_maps = _make_in_maps(inputs)
    res = bass_utils.run_bass_kernel_spmd(
        nc, in_maps, core_ids=list(range(NCORE)), trace=trace
    )
    yfull = np.concatenate([res.results[i]["y"] for i in range(NCORE)], axis=0)
    return yfull.reshape(4, 2048, D), res


def kernel(**inputs) -> np.ndarray:
    y, _ = run(inputs, trace=False)
    return y


# revision 8
# speedup vs baseline: 1.1810x; 1.1810x over previous
"""nn_MoEMLP — Trainium2 Bass kernel (8 NeuronCores, expert-parallel).

kernel(**inputs) takes the FULL unsharded inputs (as produced by
setup_inputs) and returns the FULL output [4, 2048, 1024] fp32.

Strategy (per core i == expert i, one SPMD program):
  - fp32 router on the core's 1024-token shard (logits -> softmax -> top-2)
  - AllGather of the per-shard top-2 masks -> global mask [8192, 8]
  - global positions via per-row prefix scans + triangular matmuls;
    capacity masking (C=1280); a fully static slot->token inversion built
    from one-hot selection against kept-row prefixes (no data-dependent
    scatter)
  - dispatch: 10 indirect row-gathers of x, PE-transpose to bufT [1024, 1280]
  - expert FFN in bf16: hT = gelu(w1.T @ bufT + b1); out = hT.T @ w2 + b2
  - AllGather of expert outputs (bf16) -> [10240, 1024]
  - combine: per-token weighted sum of its two expert rows via indirect
    gathers; each core emits its token shard of y; host concatenates.
"""
import numpy as np
from contextlib import ExitStack

import concourse.bass as bass
import concourse.mybir as mybir
import concourse.tile as tile
from concourse import bacc, bass_utils

F32 = mybir.dt.float32
BF16 = mybir.dt.bfloat16
I32 = mybir.dt.int32
U32 = mybir.dt.uint32
AF = mybir.ActivationFunctionType
OP = mybir.AluOpType

P = 128
D = 1024
DH = 4096
E = 8
NCORE = 8
NTOK = 8192
TSH = 1024
CAP = 1280
JW = 64
CB = 256
NCB = 5
SC = 10

_CACHE = {}


def _build():
    nc = bacc.Bacc("TRN2", target_bir_lowering=False, debug=False, num_devices=NCORE)

    x = nc.dram_tensor("x", [NTOK, D], F32, kind="ExternalInput")
    xs = nc.dram_tensor("xs", [TSH, D], F32, kind="ExternalInput")
    rw = nc.dram_tensor("rw", [D, E], F32, kind="ExternalInput")
    rb = nc.dram_tensor("rb", [1, E], F32, kind="ExternalInput")
    w1 = nc.dram_tensor("w1", [D, DH], BF16, kind="ExternalInput")
    w2 = nc.dram_tensor("w2", [DH, D], BF16, kind="ExternalInput")
    b1 = nc.dram_tensor("b1", [1, DH], F32, kind="ExternalInput")
    b2 = nc.dram_tensor("b2", [1, D], F32, kind="ExternalInput")
    ohcol = nc.dram_tensor("ohcol", [P, E], F32, kind="ExternalInput")
    sel16 = nc.dram_tensor("sel16", [P, 1], F32, kind="ExternalInput")
    y = nc.dram_tensor("y", [TSH, D], F32, kind="ExternalOutput")

    agin1 = nc.dram_tensor("agin1", [TSH, E], F32, kind="Internal")
    gmask = nc.dram_tensor("gmask", [NTOK, E], F32, kind="Internal", addr_space="Shared")
    agin2 = nc.dram_tensor("agin2", [CAP, D], BF16, kind="Internal")
    outall = nc.dram_tensor("outall", [E * CAP, D], BF16, kind="Internal", addr_space="Shared")

    ident_f = nc.inline_tensor(np.eye(P, dtype=np.float32), "ident_f")
    tri_np = (np.arange(P)[:, None] < np.arange(P)[None, :]).astype(np.float32)
    tril = nc.inline_tensor(tri_np, "tril")
    ones_1x128 = nc.inline_tensor(np.ones((1, P), np.float32), "ones_1x128")
    ones_128x1 = nc.inline_tensor(np.ones((P, 1), np.float32), "ones_128x1")
    ones_64x1 = nc.inline_tensor(np.ones((64, 1), np.float32), "ones_64x1")
    ones_1x64 = nc.inline_tensor(np.ones((1, 64), np.float32), "ones_1x64")
    iota_sf = nc.inline_tensor(
        np.broadcast_to(np.arange(P, dtype=np.float32), (P, P)).copy(), "iota_sf")
    iota_rp = nc.inline_tensor(
        np.broadcast_to(np.arange(64, dtype=np.float32)[:, None], (64, P)).copy(), "iota_rp")
    iota_row = nc.inline_tensor(np.arange(P, dtype=np.float32)[None, :].copy(), "iota_row")
    keysraw = nc.inline_tensor(
        np.broadcast_to((JW - np.arange(JW)).astype(np.float32), (P, JW)).copy(), "keysraw")
    iota_e1280 = nc.inline_tensor(
        np.broadcast_to((np.arange(E) * CAP).astype(np.float32), (P, E)).copy(), "iota_e1280")
    pval = nc.inline_tensor(np.arange(P, dtype=np.float32)[:, None].copy(), "pval")

    with tile.TileContext(nc) as tc, ExitStack() as ctx:
        pp = ctx.enter_context(tc.tile_pool(name="persist", bufs=1))
        wk = ctx.enter_context(tc.tile_pool(name="work", bufs=2))
        psT = ctx.enter_context(tc.tile_pool(name="psT", bufs=2, space="PSUM"))
        psS = ctx.enter_context(tc.tile_pool(name="psS", bufs=2, space="PSUM"))
        ps1p = ctx.enter_context(tc.tile_pool(name="ps1p", bufs=2, space="PSUM"))
        ps2p = ctx.enter_context(tc.tile_pool(name="ps2p", bufs=1, space="PSUM"))

        def t(pool, shape, dt, tag, bufs=None):
            if bufs is None:
                return pool.tile(shape, dt, tag=tag, name=tag)
            return pool.tile(shape, dt, tag=tag, name=tag, bufs=bufs)

        idf = t(pp, [P, P], F32, "idf"); nc.sync.dma_start(idf[:], ident_f.ap())
        trl = t(pp, [P, P], F32, "trl"); nc.sync.dma_start(trl[:], tril.ap())
        o1x128 = t(pp, [1, P], F32, "o1x128"); nc.sync.dma_start(o1x128[:], ones_1x128.ap())
        o128x1 = t(pp, [P, 1], F32, "o128x1"); nc.sync.dma_start(o128x1[:], ones_128x1.ap())
        o64x1 = t(pp, [64, 1], F32, "o64x1"); nc.sync.dma_start(o64x1[:], ones_64x1.ap())
        o1x64 = t(pp, [1, 64], F32, "o1x64"); nc.sync.dma_start(o1x64[:], ones_1x64.ap())
        isf = t(pp, [P, P], F32, "isf"); nc.sync.dma_start(isf[:], iota_sf.ap())
        irp = t(pp, [64, P], F32, "irp"); nc.sync.dma_start(irp[:], iota_rp.ap())
        irow = t(pp, [1, P], F32, "irow"); nc.sync.dma_start(irow[:], iota_row.ap())
        kraw = t(pp, [P, JW], F32, "kraw"); nc.sync.dma_start(kraw[:], keysraw.ap())
        ie1280 = t(pp, [P, E], F32, "ie1280"); nc.sync.dma_start(ie1280[:], iota_e1280.ap())
        pvl = t(pp, [P, 1], F32, "pvl"); nc.sync.dma_start(pvl[:], pval.ap())
        ohc = t(pp, [P, E], F32, "ohc"); nc.sync.dma_start(ohc[:], ohcol.ap())
        s16 = t(pp, [P, 1], F32, "s16"); nc.sync.dma_start(s16[:], sel16.ap())
        rb_sb = t(pp, [1, E], F32, "rb_sb"); nc.sync.dma_start(rb_sb[:], rb.ap())
        rw_sb = t(pp, [P, 8 * E], F32, "rw_sb")
        for k in range(8):
            nc.sync.dma_start(rw_sb[:, k * E : (k + 1) * E], rw.ap()[k * P : (k + 1) * P, :])
        b2_sb = t(pp, [1, D], F32, "b2_sb"); nc.sync.dma_start(b2_sb[:], b2.ap())
        b1r = t(wk, [32, P], F32, "b1r")
        nc.sync.dma_start(b1r[:], b1.ap().rearrange("o (m p) -> (o m) p", p=P))
        b1p = psT.tile([P, 32], F32, tag="tp")
        nc.tensor.transpose(out=b1p[:], in_=b1r[:], identity=idf[:32, :32])
        b1t = t(pp, [P, 32], F32, "b1t")
        nc.vector.tensor_copy(b1t[:], b1p[:])

        gates = [t(pp, [P, E], F32, f"gates{c}") for c in range(8)]
        lmask = [t(pp, [P, E], F32, f"lmask{c}") for c in range(8)]
        repl1 = [t(pp, [P, E], F32, f"repl1{c}") for c in range(8)]
        repl2 = [t(pp, [P, E], F32, f"repl2{c}") for c in range(8)]
        rpr = t(pp, [P, E], F32, "rpr")
        rkp = t(pp, [P, 1], F32, "rkp")
        rkpn = t(pp, [P, 1], F32, "rkpn")
        idxc = [t(pp, [P, 1], I32, f"idxc{S}") for S in range(SC)]

        # ---------------- Phase R: router ----------------
        with tc.tile_pool(name="rpool", bufs=1) as rp:
            xT = [rp.tile([P, TSH], F32, tag=f"xT{k}", name=f"xT{k}") for k in range(8)]
            for c in range(8):
                xsc = t(wk, [P, D], F32, "big4k", bufs=4)
                nc.sync.dma_start(xsc[:], xs.ap()[c * P : (c + 1) * P, :])
                for k in range(8):
                    tp = psT.tile([P, P], F32, tag="tp")
                    nc.tensor.transpose(out=tp[:], in_=xsc[:, k * P : (k + 1) * P], identity=idf[:])
                    nc.vector.tensor_copy(xT[k][:, c * P : (c + 1) * P], tp[:])
            for c in range(8):
                lg = psS.tile([P, E], F32, tag="ss")
                nc.tensor.matmul(lg[:], lhsT=o1x128[:], rhs=rb_sb[:], start=True, stop=False)
                for k in range(8):
                    nc.tensor.matmul(
                        lg[:], lhsT=xT[k][:, c * P : (c + 1) * P],
                        rhs=rw_sb[:, k * E : (k + 1) * E], start=False, stop=(k == 7))
                l_sb = t(wk, [P, E], F32, "l_sb")
                nc.vector.tensor_copy(l_sb[:], lg[:])
                m1 = t(wk, [P, 1], F32, "m1")
                nc.vector.reduce_max(m1[:], l_sb[:], axis=mybir.AxisListType.X)
                nm = t(wk, [P, 1], F32, "nm")
                nc.vector.tensor_scalar_mul(nm[:], m1[:], -1.0)
                ex = t(wk, [P, E], F32, "ex")
                ssum = t(wk, [P, 1], F32, "ssum")
                nc.scalar.activation(ex[:], l_sb[:], AF.Exp, bias=nm[:, :1], accum_out=ssum[:, :1])
                rcp = t(wk, [P, 1], F32, "rcp")
                nc.vector.reciprocal(rcp[:], ssum[:])
                nc.vector.tensor_scalar_mul(gates[c][:], ex[:], rcp[:, :1])
                mx = t(wk, [P, E], F32, "mx")
                nc.vector.max(out=mx[:], in_=gates[c][:])
                t1r = t(wk, [P, E], F32, "t1r")
                nc.vector.tensor_copy(t1r[:], mx[:])
                nc.vector.memset(t1r[:, 1:E], -2.0)
                nc.vector.match_replace(out=repl1[c][:], in_to_replace=t1r[:], in_values=gates[c][:], imm_value=-3.0)
                t2r = t(wk, [P, E], F32, "t2r")
                nc.vector.tensor_copy(t2r[:, 0:1], mx[:, 1:2])
                nc.vector.memset(t2r[:, 1:E], -2.0)
                nc.vector.match_replace(out=repl2[c][:], in_to_replace=t2r[:], in_values=repl1[c][:], imm_value=-3.0)
                nc.vector.tensor_tensor(out=lmask[c][:], in0=gates[c][:], in1=repl2[c][:], op=OP.is_gt)
                nc.sync.dma_start(agin1.ap()[c * P : (c + 1) * P, :], lmask[c][:])

        nc.gpsimd.collective_compute(
            "AllGather", OP.bypass, replica_groups=[list(range(NCORE))],
            ins=[agin1.ap()], outs=[gmask.ap()])

        # ---------------- Phase P + D ----------------
        bufT = [t(pp, [P, CAP], BF16, f"bufT{k}") for k in range(8)]
        with tc.tile_pool(name="ppool", bufs=1) as pq:
            gm = pq.tile([P, 512], F32, tag="gm", name="gm")
            nc.sync.dma_start(gm[:], gmask.ap().rearrange("(p j) e -> p (j e)", p=P))
            z512 = pq.tile([P, 512], F32, tag="z512", name="z512")
            nc.vector.memset(z512[:], 0.0)
            sc_all = pq.tile([P, 512], F32, tag="sc_all", name="sc_all")
            for e in range(E):
                nc.vector.tensor_tensor_scan(
                    out=sc_all[:, e::E], data0=gm[:, e::E], data1=z512[:, e::E],
                    initial=-1.0, op0=OP.add, op1=OP.add)
            rtot = t(wk, [P, E], F32, "rtot")
            nc.vector.tensor_scalar_add(rtot[:], sc_all[:, 504:512], 1.0)
            rprp = psS.tile([P, E], F32, tag="ss")
            nc.tensor.matmul(rprp[:], lhsT=trl[:], rhs=rtot[:], start=True, stop=True)
            nc.vector.tensor_copy(rpr[:], rprp[:])
            for e in range(E):
                nc.vector.tensor_scalar_add(sc_all[:, e::E], sc_all[:, e::E], rpr[:, e : e + 1])
            pen = pq.tile([P, 512], F32, tag="pen", name="pen")
            nc.vector.tensor_scalar(
                out=pen[:], in0=gm[:], scalar1=-1.0e6, scalar2=1.0e6, op0=OP.mult, op1=OP.add)
            pdm = pq.tile([P, 512], F32, tag="pdm", name="pdm")
            nc.vector.tensor_add(pdm[:], sc_all[:], pen[:])
            pd_e = pq.tile([P, JW], F32, tag="pd_e", name="pd_e")
            tsel = t(wk, [P, JW], F32, "tsel")
            nc.vector.tensor_scalar_mul(pd_e[:], pdm[:, 0::E], ohc[:, 0:1])
            for k in range(1, E):
                nc.vector.tensor_scalar_mul(tsel[:], pdm[:, k::E], ohc[:, k : k + 1])
                nc.vector.tensor_add(pd_e[:], pd_e[:], tsel[:])
            kept = pq.tile([P, JW], F32, tag="kept", name="kept")
            nc.vector.tensor_scalar(
                out=kept[:], in0=pd_e[:], scalar1=float(CAP) - 0.5, scalar2=None, op0=OP.is_le)
            keyA = pq.tile([P, JW], F32, tag="keyA", name="keyA")
            keyB = pq.tile([P, JW], F32, tag="keyB", name="keyB")
            nc.vector.tensor_mul(keyA[:], kraw[:], kept[:])
            ktmp = t(wk, [P, JW], F32, "ktmp")
            nc.vector.tensor_scalar_add(ktmp[:], kept[:], -1.0)
            nc.vector.tensor_add(keyA[:], keyA[:], ktmp[:])
            loc = pq.tile([P, JW], F32, tag="loc", name="loc")
            cur, nxt = keyA, keyB
            for r8 in range(8):
                mx8 = t(wk, [P, 8], F32, "mx8")
                nc.vector.max(out=mx8[:], in_=cur[:])
                mi8 = t(wk, [P, 8], U32, "mi8")
                nc.vector.max_index(out=mi8[:], in_max=mx8[:], in_values=cur[:])
                nc.vector.tensor_copy(loc[:, r8 * 8 : (r8 + 1) * 8], mi8[:])
                if r8 < 7:
                    nc.vector.match_replace(out=nxt[:], in_to_replace=mx8[:], in_values=cur[:], imm_value=-1.0)
                    cur, nxt = nxt, cur
            rcnt = t(wk, [P, 1], F32, "rcnt")
            nc.vector.reduce_sum(rcnt[:], kept[:], axis=mybir.AxisListType.X)
            rkpp = psS.tile([P, 1], F32, tag="ss")
            nc.tensor.matmul(rkpp[:], lhsT=trl[:], rhs=rcnt[:], start=True, stop=True)
            nc.vector.tensor_copy(rkp[:], rkpp[:])
            nc.vector.tensor_add(rkpn[:], rkp[:], rcnt[:])

            for S in range(SC):
                rpS = t(wk, [P, 1], F32, "rpS")
                nc.vector.tensor_scalar_add(rpS[:], rkp[:], float(-P * S))
                rpnS = t(wk, [P, 1], F32, "rpnS")
                nc.vector.tensor_scalar_add(rpnS[:], rkpn[:], float(-P * S))
                selA = t(wk, [P, P], F32, "selA")
                nc.vector.tensor_scalar(out=selA[:], in0=isf[:], scalar1=rpS[:, :1], scalar2=None, op0=OP.is_ge)
                selB = t(wk, [P, P], F32, "selB")
                nc.vector.tensor_scalar(out=selB[:], in0=isf[:], scalar1=rpnS[:, :1], scalar2=None, op0=OP.is_lt)
                selO = t(wk, [P, P], F32, "selO")
                nc.vector.tensor_mul(selO[:], selA[:], selB[:])
                rap = psS.tile([1, P], F32, tag="ss")
                nc.tensor.matmul(rap[:], lhsT=rkp[:], rhs=selO[:], start=True, stop=True)
                psp = psS.tile([1, P], F32, tag="ss")
                nc.tensor.matmul(psp[:], lhsT=pvl[:], rhs=selO[:], start=True, stop=True)
                pst = t(wk, [1, P], F32, "pst")
                nc.vector.tensor_copy(pst[:], psp[:])
                rsr = t(wk, [1, P], F32, "rsr")
                nc.vector.tensor_scalar_add(rsr[:], irow[:], float(P * S))
                nc.vector.tensor_sub(rsr[:], rsr[:], rap[:])
                Tp = psS.tile([64, P], F32, tag="ss")
                nc.tensor.matmul(Tp[:], lhsT=loc[:], rhs=selO[:], start=True, stop=True)
                repp = psS.tile([64, P], F32, tag="ss")
                nc.tensor.matmul(repp[:], lhsT=o1x64[:], rhs=rsr[:], start=True, stop=True)
                Rm = t(wk, [64, P], F32, "Rm")
                nc.vector.tensor_tensor(out=Rm[:], in0=irp[:], in1=repp[:], op=OP.is_equal)
                RT = t(wk, [64, P], F32, "RT")
                nc.vector.tensor_mul(RT[:], Rm[:], Tp[:])
                srow = psS.tile([1, P], F32, tag="ss")
                nc.tensor.matmul(srow[:], lhsT=o64x1[:], rhs=RT[:], start=True, stop=True)
                idxr = t(wk, [1, P], F32, "idxr")
                nc.vector.tensor_scalar_mul(idxr[:], pst[:], float(JW))
                nc.vector.tensor_add(idxr[:], idxr[:], srow[:])
                itp = psS.tile([P, 1], F32, tag="ss")
                nc.tensor.transpose(out=itp[:], in_=idxr[:], identity=idf[:1, :1])
                nc.vector.tensor_copy(idxc[S][:], itp[:])

            for S in range(SC):
                xg = t(wk, [P, D], F32, "big4k", bufs=4)
                nc.gpsimd.indirect_dma_start(
                    out=xg[:], out_offset=None, in_=x.ap(),
                    in_offset=bass.IndirectOffsetOnAxis(ap=idxc[S][:, :1], axis=0))
                for k in range(8):
                    tp = psT.tile([P, P], F32, tag="tp")
                    nc.tensor.transpose(out=tp[:], in_=xg[:, k * P : (k + 1) * P], identity=idf[:])
                    nc.vector.tensor_copy(bufT[k][:, S * P : (S + 1) * P], tp[:])

        # ---------------- Phase G: expert FFN ----------------
        with tc.tile_pool(name="fpool", bufs=1) as fp:
            w1sb = [fp.tile([P, DH], BF16, tag=f"w1sb{k}", name=f"w1sb{k}") for k in range(8)]
            for k in range(8):
                nc.sync.dma_start(w1sb[k][:], w1.ap()[k * P : (k + 1) * P, :])
            # resident half of w2 (d 0:512); the other half streams per block
            w2r = [fp.tile([P, 512], BF16, tag=f"w2r{h}", name=f"w2r{h}") for h in range(32)]
            for h in range(32):
                nc.sync.dma_start(w2r[h][:], w2.ap()[h * P : (h + 1) * P, 0:512])
            hT = fp.tile([P, 32 * CB], BF16, tag="hT", name="hT")
            for cb in range(NCB):
                c0 = cb * CB
                for m in range(32):
                    ps1 = ps1p.tile([P, CB], F32, tag="ps1")
                    for k in range(8):
                        nc.tensor.matmul(
                            ps1[:], lhsT=w1sb[k][:, m * P : (m + 1) * P],
                            rhs=bufT[k][:, c0 : c0 + CB], start=(k == 0), stop=(k == 7))
                    nc.scalar.activation(
                        hT[:, m * CB : (m + 1) * CB], ps1[:], AF.Gelu, bias=b1t[:, m : m + 1])
                orow = t(wk, [P, D], BF16, "big2k", bufs=4)
                orow2 = t(wk, [P, D], BF16, "big2k", bufs=4)
                ot = (orow, orow2)
                w2s_tiles = []
                for h in range(32):
                    w2h = fp.tile([P, 512], BF16, tag="w2h", name="w2h", bufs=29)
                    nc.sync.dma_start(w2h[:], w2.ap()[h * P : (h + 1) * P, 512:1024])
                    w2s_tiles.append(w2h)
                for dn in range(2):
                    ps2a = ps2p.tile([P, 512], F32, tag="ps2a")
                    ps2b = ps2p.tile([P, 512], F32, tag="ps2b")
                    pt2 = (ps2a, ps2b)
                    for cc in range(2):
                        nc.tensor.matmul(
                            pt2[cc][:], lhsT=o1x128[:], rhs=b2_sb[:, dn * 512 : (dn + 1) * 512],
                            start=True, stop=False)
                    for h in range(32):
                        rhs = w2r[h] if dn == 0 else w2s_tiles[h]
                        for cc in range(2):
                            nc.tensor.matmul(
                                pt2[cc][:], lhsT=hT[:, h * CB + cc * P : h * CB + (cc + 1) * P],
                                rhs=rhs[:], start=False, stop=(h == 31))
                    for cc in range(2):
                        nc.vector.tensor_copy(ot[cc][:, dn * 512 : (dn + 1) * 512], pt2[cc][:])
                for cc in range(2):
                    nc.sync.dma_start(agin2.ap()[c0 + cc * P : c0 + (cc + 1) * P, :], ot[cc][:])

        nc.gpsimd.collective_compute(
            "AllGather", OP.bypass, replica_groups=[list(range(NCORE))],
            ins=[agin2.ap()], outs=[outall.ap()])

        # ---------------- Phase C: combine ----------------
        basep = psS.tile([1, E], F32, tag="ss")
        nc.tensor.matmul(basep[:], lhsT=s16[:], rhs=rpr[:], start=True, stop=True)
        cum = t(wk, [1, E], F32, "cum0")
        nc.vector.tensor_copy(cum[:], basep[:])
        for c in range(8):
            lpp = psS.tile([P, E], F32, tag="ss")
            nc.tensor.matmul(lpp[:], lhsT=o1x128[:], rhs=cum[:], start=True, stop=False)
            nc.tensor.matmul(lpp[:], lhsT=trl[:], rhs=lmask[c][:], start=False, stop=True)
            lp = t(wk, [P, E], F32, "lp")
            nc.vector.tensor_copy(lp[:], lpp[:])
            if c < 7:
                totp = psS.tile([1, E], F32, tag="ss")
                nc.tensor.matmul(totp[:], lhsT=o128x1[:], rhs=lmask[c][:], start=True, stop=True)
                ncum = t(wk, [1, E], F32, "cumN")
                nc.vector.tensor_add(ncum[:], cum[:], totp[:])
                cum = ncum
            va = t(wk, [P, E], F32, "va")
            nc.vector.tensor_scalar(out=va[:], in0=lp[:], scalar1=float(CAP) - 0.5, scalar2=None, op0=OP.is_le)
            lval = t(wk, [P, E], F32, "lval")
            nc.vector.tensor_mul(lval[:], va[:], lmask[c][:])
            lpc = t(wk, [P, E], F32, "lpc")
            nc.vector.tensor_scalar_min(lpc[:], lp[:], float(CAP - 1))
            rowid = t(wk, [P, E], F32, "rowid")
            nc.vector.tensor_add(rowid[:], lpc[:], ie1280[:])
            gv = t(wk, [P, E], F32, "gv")
            nc.vector.tensor_mul(gv[:], gates[c][:], lval[:])
            is1 = t(wk, [P, E], F32, "is1")
            nc.vector.tensor_tensor(out=is1[:], in0=gates[c][:], in1=repl1[c][:], op=OP.is_gt)
            is2 = t(wk, [P, E], F32, "is2")
            nc.vector.tensor_tensor(out=is2[:], in0=repl1[c][:], in1=repl2[c][:], op=OP.is_gt)
            junk = t(wk, [P, E], F32, "junk")
            w1s = t(wk, [P, 1], F32, "w1s")
            nc.vector.tensor_mul(junk[:], is1[:], gv[:])
            nc.vector.reduce_sum(w1s[:], junk[:], axis=mybir.AxisListType.X)
            w2s = t(wk, [P, 1], F32, "w2s")
            nc.vector.tensor_mul(junk[:], is2[:], gv[:])
            nc.vector.reduce_sum(w2s[:], junk[:], axis=mybir.AxisListType.X)
            g1f = t(wk, [P, 1], F32, "g1f")
            nc.vector.tensor_mul(junk[:], is1[:], rowid[:])
            nc.vector.reduce_sum(g1f[:], junk[:], axis=mybir.AxisListType.X)
            g2f = t(wk, [P, 1], F32, "g2f")
            nc.vector.tensor_mul(junk[:], is2[:], rowid[:])
            nc.vector.reduce_sum(g2f[:], junk[:], axis=mybir.AxisListType.X)
            g1i = t(wk, [P, 1], I32, "g1i")
            nc.vector.tensor_copy(g1i[:], g1f[:])
            g2i = t(wk, [P, 1], I32, "g2i")
            nc.vector.tensor_copy(g2i[:], g2f[:])
            r1 = t(wk, [P, D], BF16, "big2k", bufs=4)
            nc.gpsimd.indirect_dma_start(
                out=r1[:], out_offset=None, in_=outall.ap(),
                in_offset=bass.IndirectOffsetOnAxis(ap=g1i[:, :1], axis=0))
            r2 = t(wk, [P, D], BF16, "big2k", bufs=4)
            nc.gpsimd.indirect_dma_start(
                out=r2[:], out_offset=None, in_=outall.ap(),
                in_offset=bass.IndirectOffsetOnAxis(ap=g2i[:, :1], axis=0))
            y1 = t(wk, [P, D], F32, "big4k", bufs=4)
            nc.vector.tensor_scalar_mul(y1[:], r1[:], w1s[:, :1])
            y2 = t(wk, [P, D], F32, "big4k", bufs=4)
            nc.scalar.activation(y2[:], r2[:], AF.Copy, scale=w2s[:, :1])
            yc = t(wk, [P, D], F32, "big4k", bufs=4)
            nc.vector.tensor_add(yc[:], y1[:], y2[:])
            nc.sync.dma_start(y.ap()[c * P : (c + 1) * P, :], yc[:])

    nc.compile()
    return nc


def _make_in_maps(inputs):
    import ml_dtypes

    x = np.ascontiguousarray(np.asarray(inputs["x"], np.float32).reshape(NTOK, D))
    rw = np.ascontiguousarray(np.asarray(inputs["router_w"], np.float32))
    rb = np.ascontiguousarray(np.asarray(inputs["router_b"], np.float32)).reshape(1, E)
    w1 = np.asarray(inputs["w1"])
    w2 = np.asarray(inputs["w2"])
    b1 = np.asarray(inputs["b1"])
    b2 = np.asarray(inputs["b2"])
    in_maps = []
    for i in range(NCORE):
        oh = np.zeros((P, E), np.float32)
        oh[:, i] = 1.0
        s16 = np.zeros((P, 1), np.float32)
        s16[16 * i, 0] = 1.0
        in_maps.append({
            "x": x,
            "xs": np.ascontiguousarray(x[i * TSH : (i + 1) * TSH]),
            "rw": rw,
            "rb": rb,
            "w1": np.ascontiguousarray(np.asarray(w1[i], np.float32).astype(ml_dtypes.bfloat16)),
            "w2": np.ascontiguousarray(np.asarray(w2[i], np.float32).astype(ml_dtypes.bfloat16)),
            "b1": np.ascontiguousarray(np.asarray(b1[i], np.float32)).reshape(1, DH),
            "b2": np.ascontiguousarray(np.asarray(b2[i], np.float32)).reshape(1, D),
            "ohcol": oh,
            "sel16": s16,
        })
    return in_maps


def run(inputs, trace=False):
    if "nc" not in _CACHE:
        _CACHE["nc"] = _build()
    nc = _CACHE["nc"]
    in_maps = _make_in_maps(inputs)
    res = bass_utils.run_bass_kernel_spmd(
        nc, in_maps, core_ids=list(range(NCORE)), trace=trace
    )
    yfull = np.concatenate([res.results[i]["y"] for i in range(NCORE)], axis=0)
    return yfull.reshape(4, 2048, D), res


def kernel(**inputs) -> np.ndarray:
    y, _ = run(inputs, trace=False)
    return y


# revision 9
# speedup vs baseline: 1.2737x; 1.0784x over previous
"""nn_MoEMLP — Trainium2 Bass kernel (8 NeuronCores, expert-parallel).

kernel(**inputs) takes the FULL unsharded inputs (as produced by
setup_inputs) and returns the FULL output [4, 2048, 1024] fp32.

Strategy (per core i == expert i, one SPMD program):
  - fp32 router on the core's 1024-token shard (logits -> softmax -> top-2)
  - AllGather of the per-shard top-2 masks -> global mask [8192, 8]
  - global positions via per-row prefix scans + triangular matmuls;
    capacity masking (C=1280); a fully static slot->token inversion built
    from one-hot selection against kept-row prefixes (no data-dependent
    scatter)
  - dispatch: 10 indirect row-gathers of x, PE-transpose to bufT [1024, 1280]
  - expert FFN in bf16: hT = gelu(w1.T @ bufT + b1); out = hT.T @ w2 + b2
  - AllGather of expert outputs (bf16) -> [10240, 1024]
  - combine: per-token weighted sum of its two expert rows via indirect
    gathers; each core emits its token shard of y; host concatenates.
"""
import numpy as np
from contextlib import ExitStack

import concourse.bass as bass
import concourse.mybir as mybir
import concourse.tile as tile
from concourse import bacc, bass_utils

F32 = mybir.dt.float32
BF16 = mybir.dt.bfloat16
I32 = mybir.dt.int32
U32 = mybir.dt.uint32
AF = mybir.ActivationFunctionType
OP = mybir.AluOpType

P = 128
D = 1024
DH = 4096
E = 8
NCORE = 8
NTOK = 8192
TSH = 1024
CAP = 1280
JW = 64
CB = 256
NCB = 5
SC = 10

_CACHE = {}


def _build():
    nc = bacc.Bacc("TRN2", target_bir_lowering=False, debug=False, num_devices=NCORE)

    x = nc.dram_tensor("x", [NTOK, D], F32, kind="ExternalInput")
    xs = nc.dram_tensor("xs", [TSH, D], F32, kind="ExternalInput")
    rw = nc.dram_tensor("rw", [D, E], F32, kind="ExternalInput")
    rb = nc.dram_tensor("rb", [1, E], F32, kind="ExternalInput")
    w1 = nc.dram_tensor("w1", [D, DH], BF16, kind="ExternalInput")
    w2 = nc.dram_tensor("w2", [DH, D], BF16, kind="ExternalInput")
    b1 = nc.dram_tensor("b1", [1, DH], F32, kind="ExternalInput")
    b2 = nc.dram_tensor("b2", [1, D], F32, kind="ExternalInput")
    ohcol = nc.dram_tensor("ohcol", [P, E], F32, kind="ExternalInput")
    sel16 = nc.dram_tensor("sel16", [P, 1], F32, kind="ExternalInput")
    y = nc.dram_tensor("y", [TSH, D], F32, kind="ExternalOutput")

    agin1 = nc.dram_tensor("agin1", [TSH, E], F32, kind="Internal")
    gmask = nc.dram_tensor("gmask", [NTOK, E], F32, kind="Internal", addr_space="Shared")
    agin2 = nc.dram_tensor("agin2", [CAP, D], BF16, kind="Internal")
    outall = nc.dram_tensor("outall", [E * CAP, D], BF16, kind="Internal", addr_space="Shared")

    ident_f = nc.inline_tensor(np.eye(P, dtype=np.float32), "ident_f")
    tri_np = (np.arange(P)[:, None] < np.arange(P)[None, :]).astype(np.float32)
    tril = nc.inline_tensor(tri_np, "tril")
    ones_1x128 = nc.inline_tensor(np.ones((1, P), np.float32), "ones_1x128")
    ones_128x1 = nc.inline_tensor(np.ones((P, 1), np.float32), "ones_128x1")
    ones_64x1 = nc.inline_tensor(np.ones((64, 1), np.float32), "ones_64x1")
    ones_1x64 = nc.inline_tensor(np.ones((1, 64), np.float32), "ones_1x64")
    iota_sf = nc.inline_tensor(
        np.broadcast_to(np.arange(P, dtype=np.float32), (P, P)).copy(), "iota_sf")
    iota_rp = nc.inline_tensor(
        np.broadcast_to(np.arange(64, dtype=np.float32)[:, None], (64, P)).copy(), "iota_rp")
    iota_row = nc.inline_tensor(np.arange(P, dtype=np.float32)[None, :].copy(), "iota_row")
    keysraw = nc.inline_tensor(
        np.broadcast_to((JW - np.arange(JW)).astype(np.float32), (P, JW)).copy(), "keysraw")
    iota_e1280 = nc.inline_tensor(
        np.broadcast_to((np.arange(E) * CAP).astype(np.float32), (P, E)).copy(), "iota_e1280")
    iota_e256 = nc.inline_tensor(
        np.broadcast_to((np.arange(E) * 256).astype(np.float32), (P, E)).copy(), "iota_e256")
    pval = nc.inline_tensor(np.arange(P, dtype=np.float32)[:, None].copy(), "pval")

    with tile.TileContext(nc) as tc, ExitStack() as ctx:
        pp = ctx.enter_context(tc.tile_pool(name="persist", bufs=1))
        wk = ctx.enter_context(tc.tile_pool(name="work", bufs=2))
        psT = ctx.enter_context(tc.tile_pool(name="psT", bufs=2, space="PSUM"))
        psS = ctx.enter_context(tc.tile_pool(name="psS", bufs=2, space="PSUM"))
        ps1p = ctx.enter_context(tc.tile_pool(name="ps1p", bufs=2, space="PSUM"))
        ps2p = ctx.enter_context(tc.tile_pool(name="ps2p", bufs=1, space="PSUM"))

        def t(pool, shape, dt, tag, bufs=None):
            if bufs is None:
                return pool.tile(shape, dt, tag=tag, name=tag)
            return pool.tile(shape, dt, tag=tag, name=tag, bufs=bufs)

        idf = t(pp, [P, P], F32, "idf"); nc.sync.dma_start(idf[:], ident_f.ap())
        trl = t(pp, [P, P], F32, "trl"); nc.sync.dma_start(trl[:], tril.ap())
        o1x128 = t(pp, [1, P], F32, "o1x128"); nc.sync.dma_start(o1x128[:], ones_1x128.ap())
        o128x1 = t(pp, [P, 1], F32, "o128x1"); nc.sync.dma_start(o128x1[:], ones_128x1.ap())
        o64x1 = t(pp, [64, 1], F32, "o64x1"); nc.sync.dma_start(o64x1[:], ones_64x1.ap())
        o1x64 = t(pp, [1, 64], F32, "o1x64"); nc.sync.dma_start(o1x64[:], ones_1x64.ap())
        isf = t(pp, [P, P], F32, "isf"); nc.sync.dma_start(isf[:], iota_sf.ap())
        irp = t(pp, [64, P], F32, "irp"); nc.sync.dma_start(irp[:], iota_rp.ap())
        irow = t(pp, [1, P], F32, "irow"); nc.sync.dma_start(irow[:], iota_row.ap())
        kraw = t(pp, [P, JW], F32, "kraw"); nc.sync.dma_start(kraw[:], keysraw.ap())
        ie1280 = t(pp, [P, E], F32, "ie1280"); nc.sync.dma_start(ie1280[:], iota_e1280.ap())
        ie256 = t(pp, [P, E], F32, "ie256"); nc.sync.dma_start(ie256[:], iota_e256.ap())
        pvl = t(pp, [P, 1], F32, "pvl"); nc.sync.dma_start(pvl[:], pval.ap())
        ohc = t(pp, [P, E], F32, "ohc"); nc.sync.dma_start(ohc[:], ohcol.ap())
        s16 = t(pp, [P, 1], F32, "s16"); nc.sync.dma_start(s16[:], sel16.ap())
        rb_sb = t(pp, [1, E], F32, "rb_sb"); nc.sync.dma_start(rb_sb[:], rb.ap())
        rw_sb = t(pp, [P, 8 * E], F32, "rw_sb")
        for k in range(8):
            nc.sync.dma_start(rw_sb[:, k * E : (k + 1) * E], rw.ap()[k * P : (k + 1) * P, :])
        b2_sb = t(pp, [1, D], F32, "b2_sb"); nc.sync.dma_start(b2_sb[:], b2.ap())
        b1r = t(wk, [32, P], F32, "b1r")
        nc.sync.dma_start(b1r[:], b1.ap().rearrange("o (m p) -> (o m) p", p=P))
        b1p = psT.tile([P, 32], F32, tag="tp")
        nc.tensor.transpose(out=b1p[:], in_=b1r[:], identity=idf[:32, :32])
        b1t = t(pp, [P, 32], F32, "b1t")
        nc.vector.tensor_copy(b1t[:], b1p[:])

        gates = [t(pp, [P, E], F32, f"gates{c}") for c in range(8)]
        lmask = [t(pp, [P, E], F32, f"lmask{c}") for c in range(8)]
        repl1 = [t(pp, [P, E], F32, f"repl1{c}") for c in range(8)]
        repl2 = [t(pp, [P, E], F32, f"repl2{c}") for c in range(8)]
        rpr = t(pp, [P, E], F32, "rpr")
        rkp = t(pp, [P, 1], F32, "rkp")
        rkpn = t(pp, [P, 1], F32, "rkpn")
        idxc = [t(pp, [P, 1], I32, f"idxc{S}") for S in range(SC)]

        # ---------------- Phase R: router ----------------
        with tc.tile_pool(name="rpool", bufs=1) as rp:
            xT = [rp.tile([P, TSH], F32, tag=f"xT{k}", name=f"xT{k}") for k in range(8)]
            for c in range(8):
                xsc = t(wk, [P, D], F32, "big4k", bufs=4)
                nc.sync.dma_start(xsc[:], xs.ap()[c * P : (c + 1) * P, :])
                for k in range(8):
                    tp = psT.tile([P, P], F32, tag="tp")
                    nc.tensor.transpose(out=tp[:], in_=xsc[:, k * P : (k + 1) * P], identity=idf[:])
                    nc.vector.tensor_copy(xT[k][:, c * P : (c + 1) * P], tp[:])
            for c in range(8):
                lg = psS.tile([P, E], F32, tag="ss")
                nc.tensor.matmul(lg[:], lhsT=o1x128[:], rhs=rb_sb[:], start=True, stop=False)
                for k in range(8):
                    nc.tensor.matmul(
                        lg[:], lhsT=xT[k][:, c * P : (c + 1) * P],
                        rhs=rw_sb[:, k * E : (k + 1) * E], start=False, stop=(k == 7))
                l_sb = t(wk, [P, E], F32, "l_sb")
                nc.vector.tensor_copy(l_sb[:], lg[:])
                m1 = t(wk, [P, 1], F32, "m1")
                nc.vector.reduce_max(m1[:], l_sb[:], axis=mybir.AxisListType.X)
                nm = t(wk, [P, 1], F32, "nm")
                nc.vector.tensor_scalar_mul(nm[:], m1[:], -1.0)
                ex = t(wk, [P, E], F32, "ex")
                ssum = t(wk, [P, 1], F32, "ssum")
                nc.scalar.activation(ex[:], l_sb[:], AF.Exp, bias=nm[:, :1], accum_out=ssum[:, :1])
                rcp = t(wk, [P, 1], F32, "rcp")
                nc.vector.reciprocal(rcp[:], ssum[:])
                nc.vector.tensor_scalar_mul(gates[c][:], ex[:], rcp[:, :1])
                mx = t(wk, [P, E], F32, "mx")
                nc.vector.max(out=mx[:], in_=gates[c][:])
                t1r = t(wk, [P, E], F32, "t1r")
                nc.vector.tensor_copy(t1r[:], mx[:])
                nc.vector.memset(t1r[:, 1:E], -2.0)
                nc.vector.match_replace(out=repl1[c][:], in_to_replace=t1r[:], in_values=gates[c][:], imm_value=-3.0)
                t2r = t(wk, [P, E], F32, "t2r")
                nc.vector.tensor_copy(t2r[:, 0:1], mx[:, 1:2])
                nc.vector.memset(t2r[:, 1:E], -2.0)
                nc.vector.match_replace(out=repl2[c][:], in_to_replace=t2r[:], in_values=repl1[c][:], imm_value=-3.0)
                nc.vector.tensor_tensor(out=lmask[c][:], in0=gates[c][:], in1=repl2[c][:], op=OP.is_gt)
                nc.sync.dma_start(agin1.ap()[c * P : (c + 1) * P, :], lmask[c][:])

        nc.gpsimd.collective_compute(
            "AllGather", OP.bypass, replica_groups=[list(range(NCORE))],
            ins=[agin1.ap()], outs=[gmask.ap()])

        # ---------------- Phase P + D ----------------
        bufT = [t(pp, [P, CAP], BF16, f"bufT{k}") for k in range(8)]
        with tc.tile_pool(name="ppool", bufs=1) as pq:
            gm = pq.tile([P, 512], F32, tag="gm", name="gm")
            nc.sync.dma_start(gm[:], gmask.ap().rearrange("(p j) e -> p (j e)", p=P))
            z512 = pq.tile([P, 512], F32, tag="z512", name="z512")
            nc.vector.memset(z512[:], 0.0)
            sc_all = pq.tile([P, 512], F32, tag="sc_all", name="sc_all")
            for e in range(E):
                nc.vector.tensor_tensor_scan(
                    out=sc_all[:, e::E], data0=gm[:, e::E], data1=z512[:, e::E],
                    initial=-1.0, op0=OP.add, op1=OP.add)
            rtot = t(wk, [P, E], F32, "rtot")
            nc.vector.tensor_scalar_add(rtot[:], sc_all[:, 504:512], 1.0)
            rprp = psS.tile([P, E], F32, tag="ss")
            nc.tensor.matmul(rprp[:], lhsT=trl[:], rhs=rtot[:], start=True, stop=True)
            nc.vector.tensor_copy(rpr[:], rprp[:])
            for e in range(E):
                nc.vector.tensor_scalar_add(sc_all[:, e::E], sc_all[:, e::E], rpr[:, e : e + 1])
            pen = pq.tile([P, 512], F32, tag="pen", name="pen")
            nc.vector.tensor_scalar(
                out=pen[:], in0=gm[:], scalar1=-1.0e6, scalar2=1.0e6, op0=OP.mult, op1=OP.add)
            pdm = pq.tile([P, 512], F32, tag="pdm", name="pdm")
            nc.vector.tensor_add(pdm[:], sc_all[:], pen[:])
            pd_e = pq.tile([P, JW], F32, tag="pd_e", name="pd_e")
            tsel = t(wk, [P, JW], F32, "tsel")
            nc.vector.tensor_scalar_mul(pd_e[:], pdm[:, 0::E], ohc[:, 0:1])
            for k in range(1, E):
                nc.vector.tensor_scalar_mul(tsel[:], pdm[:, k::E], ohc[:, k : k + 1])
                nc.vector.tensor_add(pd_e[:], pd_e[:], tsel[:])
            kept = pq.tile([P, JW], F32, tag="kept", name="kept")
            nc.vector.tensor_scalar(
                out=kept[:], in0=pd_e[:], scalar1=float(CAP) - 0.5, scalar2=None, op0=OP.is_le)
            keyA = pq.tile([P, JW], F32, tag="keyA", name="keyA")
            keyB = pq.tile([P, JW], F32, tag="keyB", name="keyB")
            nc.vector.tensor_mul(keyA[:], kraw[:], kept[:])
            ktmp = t(wk, [P, JW], F32, "ktmp")
            nc.vector.tensor_scalar_add(ktmp[:], kept[:], -1.0)
            nc.vector.tensor_add(keyA[:], keyA[:], ktmp[:])
            loc = pq.tile([P, JW], F32, tag="loc", name="loc")
            cur, nxt = keyA, keyB
            for r8 in range(8):
                mx8 = t(wk, [P, 8], F32, "mx8")
                nc.vector.max(out=mx8[:], in_=cur[:])
                mi8 = t(wk, [P, 8], U32, "mi8")
                nc.vector.max_index(out=mi8[:], in_max=mx8[:], in_values=cur[:])
                nc.vector.tensor_copy(loc[:, r8 * 8 : (r8 + 1) * 8], mi8[:])
                if r8 < 7:
                    nc.vector.match_replace(out=nxt[:], in_to_replace=mx8[:], in_values=cur[:], imm_value=-1.0)
                    cur, nxt = nxt, cur
            rcnt = t(wk, [P, 1], F32, "rcnt")
            nc.vector.reduce_sum(rcnt[:], kept[:], axis=mybir.AxisListType.X)
            rkpp = psS.tile([P, 1], F32, tag="ss")
            nc.tensor.matmul(rkpp[:], lhsT=trl[:], rhs=rcnt[:], start=True, stop=True)
            nc.vector.tensor_copy(rkp[:], rkpp[:])
            nc.vector.tensor_add(rkpn[:], rkp[:], rcnt[:])

            for S in range(SC):
                rpS = t(wk, [P, 1], F32, "rpS")
                nc.vector.tensor_scalar_add(rpS[:], rkp[:], float(-P * S))
                rpnS = t(wk, [P, 1], F32, "rpnS")
                nc.vector.tensor_scalar_add(rpnS[:], rkpn[:], float(-P * S))
                selA = t(wk, [P, P], F32, "selA")
                nc.vector.tensor_scalar(out=selA[:], in0=isf[:], scalar1=rpS[:, :1], scalar2=None, op0=OP.is_ge)
                selB = t(wk, [P, P], F32, "selB")
                nc.vector.tensor_scalar(out=selB[:], in0=isf[:], scalar1=rpnS[:, :1], scalar2=None, op0=OP.is_lt)
                selO = t(wk, [P, P], F32, "selO")
                nc.vector.tensor_mul(selO[:], selA[:], selB[:])
                rap = psS.tile([1, P], F32, tag="ss")
                nc.tensor.matmul(rap[:], lhsT=rkp[:], rhs=selO[:], start=True, stop=True)
                psp = psS.tile([1, P], F32, tag="ss")
                nc.tensor.matmul(psp[:], lhsT=pvl[:], rhs=selO[:], start=True, stop=True)
                pst = t(wk, [1, P], F32, "pst")
                nc.vector.tensor_copy(pst[:], psp[:])
                rsr = t(wk, [1, P], F32, "rsr")
                nc.vector.tensor_scalar_add(rsr[:], irow[:], float(P * S))
                nc.vector.tensor_sub(rsr[:], rsr[:], rap[:])
                Tp = psS.tile([64, P], F32, tag="ss")
                nc.tensor.matmul(Tp[:], lhsT=loc[:], rhs=selO[:], start=True, stop=True)
                repp = psS.tile([64, P], F32, tag="ss")
                nc.tensor.matmul(repp[:], lhsT=o1x64[:], rhs=rsr[:], start=True, stop=True)
                Rm = t(wk, [64, P], F32, "Rm")
                nc.vector.tensor_tensor(out=Rm[:], in0=irp[:], in1=repp[:], op=OP.is_equal)
                RT = t(wk, [64, P], F32, "RT")
                nc.vector.tensor_mul(RT[:], Rm[:], Tp[:])
                srow = psS.tile([1, P], F32, tag="ss")
                nc.tensor.matmul(srow[:], lhsT=o64x1[:], rhs=RT[:], start=True, stop=True)
                idxr = t(wk, [1, P], F32, "idxr")
                nc.vector.tensor_scalar_mul(idxr[:], pst[:], float(JW))
                nc.vector.tensor_add(idxr[:], idxr[:], srow[:])
                itp = psS.tile([P, 1], F32, tag="ss")
                nc.tensor.transpose(out=itp[:], in_=idxr[:], identity=idf[:1, :1])
                nc.vector.tensor_copy(idxc[S][:], itp[:])

            for S in range(SC):
                xg = t(wk, [P, D], F32, "big4k", bufs=4)
                nc.gpsimd.indirect_dma_start(
                    out=xg[:], out_offset=None, in_=x.ap(),
                    in_offset=bass.IndirectOffsetOnAxis(ap=idxc[S][:, :1], axis=0))
                for k in range(8):
                    tp = psT.tile([P, P], F32, tag="tp")
                    nc.tensor.transpose(out=tp[:], in_=xg[:, k * P : (k + 1) * P], identity=idf[:])
                    nc.vector.tensor_copy(bufT[k][:, S * P : (S + 1) * P], tp[:])

        # --------- combine stats (precomputed, overlaps FFN schedule) ---------
        g1i = [t(pp, [P, 1], I32, f"g1i{c}") for c in range(8)]
        g2i = [t(pp, [P, 1], I32, f"g2i{c}") for c in range(8)]
        w1sv = [t(pp, [P, 1], F32, f"w1sv{c}") for c in range(8)]
        w2sv = [t(pp, [P, 1], F32, f"w2sv{c}") for c in range(8)]
        basep = psS.tile([1, E], F32, tag="ss")
        nc.tensor.matmul(basep[:], lhsT=s16[:], rhs=rpr[:], start=True, stop=True)
        cum = t(wk, [1, E], F32, "cum0")
        nc.vector.tensor_copy(cum[:], basep[:])
        for c in range(8):
            lpp = psS.tile([P, E], F32, tag="ss")
            nc.tensor.matmul(lpp[:], lhsT=o1x128[:], rhs=cum[:], start=True, stop=False)
            nc.tensor.matmul(lpp[:], lhsT=trl[:], rhs=lmask[c][:], start=False, stop=True)
            lp = t(wk, [P, E], F32, "lp")
            nc.vector.tensor_copy(lp[:], lpp[:])
            if c < 7:
                totp = psS.tile([1, E], F32, tag="ss")
                nc.tensor.matmul(totp[:], lhsT=o128x1[:], rhs=lmask[c][:], start=True, stop=True)
                ncum = t(wk, [1, E], F32, "cumN")
                nc.vector.tensor_add(ncum[:], cum[:], totp[:])
                cum = ncum
            va = t(wk, [P, E], F32, "va")
            nc.vector.tensor_scalar(out=va[:], in0=lp[:], scalar1=float(CAP) - 0.5, scalar2=None, op0=OP.is_le)
            lval = t(wk, [P, E], F32, "lval")
            nc.vector.tensor_mul(lval[:], va[:], lmask[c][:])
            lpc = t(wk, [P, E], F32, "lpc")
            nc.vector.tensor_scalar_min(lpc[:], lp[:], float(CAP - 1))
            # chunked outall layout: row = e*256 + (pos % 256) + 2048*(pos // 256)
            #   = e*256 + pos + 1792 * ch, ch = sum_k [pos >= 256k]
            chn = t(wk, [P, E], F32, "chn")
            cht = t(wk, [P, E], F32, "cht")
            nc.vector.tensor_scalar(out=chn[:], in0=lpc[:], scalar1=255.5, scalar2=None, op0=OP.is_ge)
            for kk in (512, 768, 1024):
                nc.vector.tensor_scalar(out=cht[:], in0=lpc[:], scalar1=float(kk) - 0.5, scalar2=None, op0=OP.is_ge)
                nc.vector.tensor_add(chn[:], chn[:], cht[:])
            rowid = t(wk, [P, E], F32, "rowid")
            nc.vector.tensor_scalar_mul(rowid[:], chn[:], 1792.0)
            nc.vector.tensor_add(rowid[:], rowid[:], lpc[:])
            nc.vector.tensor_add(rowid[:], rowid[:], ie256[:])
            gv = t(wk, [P, E], F32, "gv")
            nc.vector.tensor_mul(gv[:], gates[c][:], lval[:])
            is1 = t(wk, [P, E], F32, "is1")
            nc.vector.tensor_tensor(out=is1[:], in0=gates[c][:], in1=repl1[c][:], op=OP.is_gt)
            is2 = t(wk, [P, E], F32, "is2")
            nc.vector.tensor_tensor(out=is2[:], in0=repl1[c][:], in1=repl2[c][:], op=OP.is_gt)
            junk = t(wk, [P, E], F32, "junk")
            g1f = t(wk, [P, 1], F32, "g1f")
            nc.vector.tensor_mul(junk[:], is1[:], gv[:])
            nc.vector.reduce_sum(w1sv[c][:], junk[:], axis=mybir.AxisListType.X)
            nc.vector.tensor_mul(junk[:], is2[:], gv[:])
            nc.vector.reduce_sum(w2sv[c][:], junk[:], axis=mybir.AxisListType.X)
            nc.vector.tensor_mul(junk[:], is1[:], rowid[:])
            nc.vector.reduce_sum(g1f[:], junk[:], axis=mybir.AxisListType.X)
            nc.vector.tensor_copy(g1i[c][:], g1f[:])
            nc.vector.tensor_mul(junk[:], is2[:], rowid[:])
            nc.vector.reduce_sum(g1f[:], junk[:], axis=mybir.AxisListType.X)
            nc.vector.tensor_copy(g2i[c][:], g1f[:])

        # ---------------- Phase G: expert FFN ----------------
        with tc.tile_pool(name="fpool", bufs=1) as fp:
            w1sb = [fp.tile([P, DH], BF16, tag=f"w1sb{k}", name=f"w1sb{k}") for k in range(8)]
            for k in range(8):
                nc.sync.dma_start(w1sb[k][:], w1.ap()[k * P : (k + 1) * P, :])
            # resident half of w2 (d 0:512); the other half streams per block
            w2r = [fp.tile([P, 512], BF16, tag=f"w2r{h}", name=f"w2r{h}") for h in range(32)]
            for h in range(32):
                nc.sync.dma_start(w2r[h][:], w2.ap()[h * P : (h + 1) * P, 0:512])
            hT = fp.tile([P, 32 * CB], BF16, tag="hT", name="hT")
            for cb in range(NCB):
                c0 = cb * CB
                for m in range(32):
                    ps1 = ps1p.tile([P, CB], F32, tag="ps1")
                    for k in range(8):
                        nc.tensor.matmul(
                            ps1[:], lhsT=w1sb[k][:, m * P : (m + 1) * P],
                            rhs=bufT[k][:, c0 : c0 + CB], start=(k == 0), stop=(k == 7))
                    nc.scalar.activation(
                        hT[:, m * CB : (m + 1) * CB], ps1[:], AF.Gelu, bias=b1t[:, m : m + 1])
                orow = t(wk, [P, D], BF16, "big2k", bufs=4)
                orow2 = t(wk, [P, D], BF16, "big2k", bufs=4)
                ot = (orow, orow2)
                w2s_tiles = []
                for h in range(32):
                    w2h = fp.tile([P, 512], BF16, tag="w2h", name="w2h", bufs=29)
                    nc.sync.dma_start(w2h[:], w2.ap()[h * P : (h + 1) * P, 512:1024])
                    w2s_tiles.append(w2h)
                for dn in range(2):
                    ps2a = ps2p.tile([P, 512], F32, tag="ps2a")
                    ps2b = ps2p.tile([P, 512], F32, tag="ps2b")
                    pt2 = (ps2a, ps2b)
                    for cc in range(2):
                        nc.tensor.matmul(
                            pt2[cc][:], lhsT=o1x128[:], rhs=b2_sb[:, dn * 512 : (dn + 1) * 512],
                            start=True, stop=False)
                    for h in range(32):
                        rhs = w2r[h] if dn == 0 else w2s_tiles[h]
                        for cc in range(2):
                            nc.tensor.matmul(
                                pt2[cc][:], lhsT=hT[:, h * CB + cc * P : h * CB + (cc + 1) * P],
                                rhs=rhs[:], start=False, stop=(h == 31))
                    for cc in range(2):
                        nc.vector.tensor_copy(ot[cc][:, dn * 512 : (dn + 1) * 512], pt2[cc][:])
                for cc in range(2):
                    nc.sync.dma_start(agin2.ap()[c0 + cc * P : c0 + (cc + 1) * P, :], ot[cc][:])
                nc.gpsimd.collective_compute(
                    "AllGather", OP.bypass, replica_groups=[list(range(NCORE))],
                    ins=[agin2.ap()[c0 : c0 + CB, :]],
                    outs=[outall.ap()[cb * (NCORE * CB) : (cb + 1) * (NCORE * CB), :]])

        # ---------------- Phase C: combine (gathers only) ----------------
        for c in range(8):
            r1 = t(wk, [P, D], BF16, "big2k", bufs=4)
            nc.gpsimd.indirect_dma_start(
                out=r1[:], out_offset=None, in_=outall.ap(),
                in_offset=bass.IndirectOffsetOnAxis(ap=g1i[c][:, :1], axis=0))
            r2 = t(wk, [P, D], BF16, "big2k", bufs=4)
            nc.gpsimd.indirect_dma_start(
                out=r2[:], out_offset=None, in_=outall.ap(),
                in_offset=bass.IndirectOffsetOnAxis(ap=g2i[c][:, :1], axis=0))
            y1 = t(wk, [P, D], F32, "big4k", bufs=4)
            nc.vector.tensor_scalar_mul(y1[:], r1[:], w1sv[c][:, :1])
            y2 = t(wk, [P, D], F32, "big4k", bufs=4)
            nc.scalar.activation(y2[:], r2[:], AF.Copy, scale=w2sv[c][:, :1])
            yc = t(wk, [P, D], F32, "big4k", bufs=4)
            nc.vector.tensor_add(yc[:], y1[:], y2[:])
            nc.sync.dma_start(y.ap()[c * P : (c + 1) * P, :], yc[:])

    nc.compile()
    return nc


def _make_in_maps(inputs):
    import ml_dtypes

    x = np.ascontiguousarray(np.asarray(inputs["x"], np.float32).reshape(NTOK, D))
    rw = np.ascontiguousarray(np.asarray(inputs["router_w"], np.float32))
    rb = np.ascontiguousarray(np.asarray(inputs["router_b"], np.float32)).reshape(1, E)
    w1 = np.asarray(inputs["w1"])
    w2 = np.asarray(inputs["w2"])
    b1 = np.asarray(inputs["b1"])
    b2 = np.asarray(inputs["b2"])
    in_maps = []
    for i in range(NCORE):
        oh = np.zeros((P, E), np.float32)
        oh[:, i] = 1.0
        s16 = np.zeros((P, 1), np.float32)
        s16[16 * i, 0] = 1.0
        in_maps.append({
            "x": x,
            "xs": np.ascontiguousarray(x[i * TSH : (i + 1) * TSH]),
            "rw": rw,
            "rb": rb,
            "w1": np.ascontiguousarray(np.asarray(w1[i], np.float32).astype(ml_dtypes.bfloat16)),
            "w2": np.ascontiguousarray(np.asarray(w2[i], np.float32).astype(ml_dtypes.bfloat16)),
            "b1": np.ascontiguousarray(np.asarray(b1[i], np.float32)).reshape(1, DH),
            "b2": np.ascontiguousarray(np.asarray(b2[i], np.float32)).reshape(1, D),
            "ohcol": oh,
            "sel16": s16,
        })
    return in_maps


def run(inputs, trace=False):
    if "nc" not in _CACHE:
        _CACHE["nc"] = _build()
    nc = _CACHE["nc"]
    in_maps = _make_in_maps(inputs)
    res = bass_utils.run_bass_kernel_spmd(
        nc, in_maps, core_ids=list(range(NCORE)), trace=trace
    )
    yfull = np.concatenate([res.results[i]["y"] for i in range(NCORE)], axis=0)
    return yfull.reshape(4, 2048, D), res


def kernel(**inputs) -> np.ndarray:
    y, _ = run(inputs, trace=False)
    return y


# revision 10
# speedup vs baseline: 1.2825x; 1.0069x over previous
"""nn_MoEMLP — Trainium2 Bass kernel (8 NeuronCores, expert-parallel).

kernel(**inputs) takes the FULL unsharded inputs (as produced by
setup_inputs) and returns the FULL output [4, 2048, 1024] fp32.

Strategy (per core i == expert i, one SPMD program):
  - fp32 router on the core's 1024-token shard (logits -> softmax -> top-2)
  - AllGather of the per-shard top-2 masks -> global mask [8192, 8]
  - global positions via per-row prefix scans + triangular matmuls;
    capacity masking (C=1280); a fully static slot->token inversion built
    from one-hot selection against kept-row prefixes (no data-dependent
    scatter)
  - dispatch: 10 indirect row-gathers of x, PE-transpose to bufT [1024, 1280]
  - expert FFN in bf16: hT = gelu(w1.T @ bufT + b1); out = hT.T @ w2 + b2
  - AllGather of expert outputs (bf16) -> [10240, 1024]
  - combine: per-token weighted sum of its two expert rows via indirect
    gathers; each core emits its token shard of y; host concatenates.
"""
import numpy as np
from contextlib import ExitStack

import concourse.bass as bass
import concourse.mybir as mybir
import concourse.tile as tile
from concourse import bacc, bass_utils

F32 = mybir.dt.float32
BF16 = mybir.dt.bfloat16
I32 = mybir.dt.int32
U32 = mybir.dt.uint32
AF = mybir.ActivationFunctionType
OP = mybir.AluOpType

P = 128
D = 1024
DH = 4096
E = 8
NCORE = 8
NTOK = 8192
TSH = 1024
CAP = 1280
JW = 64
CB = 256
NCB = 5
SC = 10

_CACHE = {}


def _build():
    nc = bacc.Bacc("TRN2", target_bir_lowering=False, debug=False, num_devices=NCORE)

    x = nc.dram_tensor("x", [NTOK, D], F32, kind="ExternalInput")
    xs = nc.dram_tensor("xs", [TSH, D], F32, kind="ExternalInput")
    rw = nc.dram_tensor("rw", [D, E], F32, kind="ExternalInput")
    rb = nc.dram_tensor("rb", [1, E], F32, kind="ExternalInput")
    w1 = nc.dram_tensor("w1", [D, DH], BF16, kind="ExternalInput")
    w2 = nc.dram_tensor("w2", [DH, D], BF16, kind="ExternalInput")
    b1 = nc.dram_tensor("b1", [1, DH], F32, kind="ExternalInput")
    b2 = nc.dram_tensor("b2", [1, D], F32, kind="ExternalInput")
    ohcol = nc.dram_tensor("ohcol", [P, E], F32, kind="ExternalInput")
    sel16 = nc.dram_tensor("sel16", [P, 1], F32, kind="ExternalInput")
    y = nc.dram_tensor("y", [TSH, D], F32, kind="ExternalOutput")

    agin1 = nc.dram_tensor("agin1", [TSH, E], F32, kind="Internal")
    gmask = nc.dram_tensor("gmask", [NTOK, E], F32, kind="Internal", addr_space="Shared")
    agin2 = nc.dram_tensor("agin2", [CAP, D], BF16, kind="Internal")
    outall = nc.dram_tensor("outall", [E * CAP, D], BF16, kind="Internal", addr_space="Shared")

    ident_f = nc.inline_tensor(np.eye(P, dtype=np.float32), "ident_f")
    tri_np = (np.arange(P)[:, None] < np.arange(P)[None, :]).astype(np.float32)
    tril = nc.inline_tensor(tri_np, "tril")
    ones_1x128 = nc.inline_tensor(np.ones((1, P), np.float32), "ones_1x128")
    ones_128x1 = nc.inline_tensor(np.ones((P, 1), np.float32), "ones_128x1")
    ones_64x1 = nc.inline_tensor(np.ones((64, 1), np.float32), "ones_64x1")
    ones_1x64 = nc.inline_tensor(np.ones((1, 64), np.float32), "ones_1x64")
    iota_sf = nc.inline_tensor(
        np.broadcast_to(np.arange(P, dtype=np.float32), (P, P)).copy(), "iota_sf")
    iota_rp = nc.inline_tensor(
        np.broadcast_to(np.arange(64, dtype=np.float32)[:, None], (64, P)).copy(), "iota_rp")
    iota_row = nc.inline_tensor(np.arange(P, dtype=np.float32)[None, :].copy(), "iota_row")
    keysraw = nc.inline_tensor(
        np.broadcast_to((JW - np.arange(JW)).astype(np.float32), (P, JW)).copy(), "keysraw")
    iota_e1280 = nc.inline_tensor(
        np.broadcast_to((np.arange(E) * CAP).astype(np.float32), (P, E)).copy(), "iota_e1280")
    iota_e256 = nc.inline_tensor(
        np.broadcast_to((np.arange(E) * 256).astype(np.float32), (P, E)).copy(), "iota_e256")
    pval = nc.inline_tensor(np.arange(P, dtype=np.float32)[:, None].copy(), "pval")

    with tile.TileContext(nc) as tc, ExitStack() as ctx:
        pp = ctx.enter_context(tc.tile_pool(name="persist", bufs=1))
        wk = ctx.enter_context(tc.tile_pool(name="work", bufs=2))
        psT = ctx.enter_context(tc.tile_pool(name="psT", bufs=2, space="PSUM"))
        psS = ctx.enter_context(tc.tile_pool(name="psS", bufs=2, space="PSUM"))
        ps1p = ctx.enter_context(tc.tile_pool(name="ps1p", bufs=2, space="PSUM"))
        ps2p = ctx.enter_context(tc.tile_pool(name="ps2p", bufs=1, space="PSUM"))

        def t(pool, shape, dt, tag, bufs=None):
            if bufs is None:
                return pool.tile(shape, dt, tag=tag, name=tag)
            return pool.tile(shape, dt, tag=tag, name=tag, bufs=bufs)

        idf = t(pp, [P, P], F32, "idf"); nc.sync.dma_start(idf[:], ident_f.ap())
        trl = t(pp, [P, P], F32, "trl"); nc.sync.dma_start(trl[:], tril.ap())
        o1x128 = t(pp, [1, P], F32, "o1x128"); nc.sync.dma_start(o1x128[:], ones_1x128.ap())
        o128x1 = t(pp, [P, 1], F32, "o128x1"); nc.sync.dma_start(o128x1[:], ones_128x1.ap())
        o64x1 = t(pp, [64, 1], F32, "o64x1"); nc.sync.dma_start(o64x1[:], ones_64x1.ap())
        o1x64 = t(pp, [1, 64], F32, "o1x64"); nc.sync.dma_start(o1x64[:], ones_1x64.ap())
        isf = t(pp, [P, P], F32, "isf"); nc.sync.dma_start(isf[:], iota_sf.ap())
        irp = t(pp, [64, P], F32, "irp"); nc.sync.dma_start(irp[:], iota_rp.ap())
        irow = t(pp, [1, P], F32, "irow"); nc.sync.dma_start(irow[:], iota_row.ap())
        kraw = t(pp, [P, JW], F32, "kraw"); nc.sync.dma_start(kraw[:], keysraw.ap())
        ie1280 = t(pp, [P, E], F32, "ie1280"); nc.sync.dma_start(ie1280[:], iota_e1280.ap())
        ie256 = t(pp, [P, E], F32, "ie256"); nc.sync.dma_start(ie256[:], iota_e256.ap())
        pvl = t(pp, [P, 1], F32, "pvl"); nc.sync.dma_start(pvl[:], pval.ap())
        ohc = t(pp, [P, E], F32, "ohc"); nc.sync.dma_start(ohc[:], ohcol.ap())
        s16 = t(pp, [P, 1], F32, "s16"); nc.sync.dma_start(s16[:], sel16.ap())
        rb_sb = t(pp, [1, E], F32, "rb_sb"); nc.sync.dma_start(rb_sb[:], rb.ap())
        rw_sb = t(pp, [P, 8 * E], F32, "rw_sb")
        for k in range(8):
            nc.sync.dma_start(rw_sb[:, k * E : (k + 1) * E], rw.ap()[k * P : (k + 1) * P, :])
        b2_sb = t(pp, [1, D], F32, "b2_sb"); nc.sync.dma_start(b2_sb[:], b2.ap())
        b1r = t(wk, [32, P], F32, "b1r")
        nc.sync.dma_start(b1r[:], b1.ap().rearrange("o (m p) -> (o m) p", p=P))
        b1p = psT.tile([P, 32], F32, tag="tp")
        nc.tensor.transpose(out=b1p[:], in_=b1r[:], identity=idf[:32, :32])
        b1t = t(pp, [P, 32], F32, "b1t")
        nc.vector.tensor_copy(b1t[:], b1p[:])

        gates = [t(pp, [P, E], F32, f"gates{c}") for c in range(8)]
        lmask = [t(pp, [P, E], F32, f"lmask{c}") for c in range(8)]
        repl1 = [t(pp, [P, E], F32, f"repl1{c}") for c in range(8)]
        repl2 = [t(pp, [P, E], F32, f"repl2{c}") for c in range(8)]
        w1sb = [t(pp, [P, DH], BF16, f"w1sb{k}") for k in range(8)]
        for k in range(8):
            nc.sync.dma_start(w1sb[k][:], w1.ap()[k * P : (k + 1) * P, :])
        rpr = t(pp, [P, E], F32, "rpr")
        rkp = t(pp, [P, 1], F32, "rkp")
        rkpn = t(pp, [P, 1], F32, "rkpn")
        idxc = [t(pp, [P, 1], I32, f"idxc{S}") for S in range(SC)]

        # ---------------- Phase R: router ----------------
        with tc.tile_pool(name="rpool", bufs=1) as rp:
            xT = [rp.tile([P, TSH], F32, tag=f"xT{k}", name=f"xT{k}") for k in range(8)]
            for c in range(8):
                xsc = t(wk, [P, D], F32, "big4k", bufs=4)
                nc.sync.dma_start(xsc[:], xs.ap()[c * P : (c + 1) * P, :])
                for k in range(8):
                    tp = psT.tile([P, P], F32, tag="tp")
                    nc.tensor.transpose(out=tp[:], in_=xsc[:, k * P : (k + 1) * P], identity=idf[:])
                    nc.vector.tensor_copy(xT[k][:, c * P : (c + 1) * P], tp[:])
            for c in range(8):
                lg = psS.tile([P, E], F32, tag="ss")
                nc.tensor.matmul(lg[:], lhsT=o1x128[:], rhs=rb_sb[:], start=True, stop=False)
                for k in range(8):
                    nc.tensor.matmul(
                        lg[:], lhsT=xT[k][:, c * P : (c + 1) * P],
                        rhs=rw_sb[:, k * E : (k + 1) * E], start=False, stop=(k == 7))
                l_sb = t(wk, [P, E], F32, "l_sb")
                nc.vector.tensor_copy(l_sb[:], lg[:])
                m1 = t(wk, [P, 1], F32, "m1")
                nc.vector.reduce_max(m1[:], l_sb[:], axis=mybir.AxisListType.X)
                nm = t(wk, [P, 1], F32, "nm")
                nc.vector.tensor_scalar_mul(nm[:], m1[:], -1.0)
                ex = t(wk, [P, E], F32, "ex")
                ssum = t(wk, [P, 1], F32, "ssum")
                nc.scalar.activation(ex[:], l_sb[:], AF.Exp, bias=nm[:, :1], accum_out=ssum[:, :1])
                rcp = t(wk, [P, 1], F32, "rcp")
                nc.vector.reciprocal(rcp[:], ssum[:])
                nc.vector.tensor_scalar_mul(gates[c][:], ex[:], rcp[:, :1])
                mx = t(wk, [P, E], F32, "mx")
                nc.vector.max(out=mx[:], in_=gates[c][:])
                t1r = t(wk, [P, E], F32, "t1r")
                nc.vector.tensor_copy(t1r[:], mx[:])
                nc.vector.memset(t1r[:, 1:E], -2.0)
                nc.vector.match_replace(out=repl1[c][:], in_to_replace=t1r[:], in_values=gates[c][:], imm_value=-3.0)
                t2r = t(wk, [P, E], F32, "t2r")
                nc.vector.tensor_copy(t2r[:, 0:1], mx[:, 1:2])
                nc.vector.memset(t2r[:, 1:E], -2.0)
                nc.vector.match_replace(out=repl2[c][:], in_to_replace=t2r[:], in_values=repl1[c][:], imm_value=-3.0)
                nc.vector.tensor_tensor(out=lmask[c][:], in0=gates[c][:], in1=repl2[c][:], op=OP.is_gt)
                nc.sync.dma_start(agin1.ap()[c * P : (c + 1) * P, :], lmask[c][:])

        nc.gpsimd.collective_compute(
            "AllGather", OP.bypass, replica_groups=[list(range(NCORE))],
            ins=[agin1.ap()], outs=[gmask.ap()])

        # ---------------- Phase P + D ----------------
        bufT = [t(pp, [P, CAP], BF16, f"bufT{k}") for k in range(8)]
        with tc.tile_pool(name="ppool", bufs=1) as pq:
            gm = pq.tile([P, 512], F32, tag="gm", name="gm")
            nc.sync.dma_start(gm[:], gmask.ap().rearrange("(p j) e -> p (j e)", p=P))
            z512 = pq.tile([P, 512], F32, tag="z512", name="z512")
            nc.vector.memset(z512[:], 0.0)
            sc_all = pq.tile([P, 512], F32, tag="sc_all", name="sc_all")
            for e in range(E):
                nc.vector.tensor_tensor_scan(
                    out=sc_all[:, e::E], data0=gm[:, e::E], data1=z512[:, e::E],
                    initial=-1.0, op0=OP.add, op1=OP.add)
            rtot = t(wk, [P, E], F32, "rtot")
            nc.vector.tensor_scalar_add(rtot[:], sc_all[:, 504:512], 1.0)
            rprp = psS.tile([P, E], F32, tag="ss")
            nc.tensor.matmul(rprp[:], lhsT=trl[:], rhs=rtot[:], start=True, stop=True)
            nc.vector.tensor_copy(rpr[:], rprp[:])
            for e in range(E):
                nc.vector.tensor_scalar_add(sc_all[:, e::E], sc_all[:, e::E], rpr[:, e : e + 1])
            pen = pq.tile([P, 512], F32, tag="pen", name="pen")
            nc.vector.tensor_scalar(
                out=pen[:], in0=gm[:], scalar1=-1.0e6, scalar2=1.0e6, op0=OP.mult, op1=OP.add)
            pdm = pq.tile([P, 512], F32, tag="pdm", name="pdm")
            nc.vector.tensor_add(pdm[:], sc_all[:], pen[:])
            pd_e = pq.tile([P, JW], F32, tag="pd_e", name="pd_e")
            tsel = t(wk, [P, JW], F32, "tsel")
            nc.vector.tensor_scalar_mul(pd_e[:], pdm[:, 0::E], ohc[:, 0:1])
            for k in range(1, E):
                nc.vector.tensor_scalar_mul(tsel[:], pdm[:, k::E], ohc[:, k : k + 1])
                nc.vector.tensor_add(pd_e[:], pd_e[:], tsel[:])
            kept = pq.tile([P, JW], F32, tag="kept", name="kept")
            nc.vector.tensor_scalar(
                out=kept[:], in0=pd_e[:], scalar1=float(CAP) - 0.5, scalar2=None, op0=OP.is_le)
            keyA = pq.tile([P, JW], F32, tag="keyA", name="keyA")
            keyB = pq.tile([P, JW], F32, tag="keyB", name="keyB")
            nc.vector.tensor_mul(keyA[:], kraw[:], kept[:])
            ktmp = t(wk, [P, JW], F32, "ktmp")
            nc.vector.tensor_scalar_add(ktmp[:], kept[:], -1.0)
            nc.vector.tensor_add(keyA[:], keyA[:], ktmp[:])
            loc = pq.tile([P, JW], F32, tag="loc", name="loc")
            cur, nxt = keyA, keyB
            for r8 in range(8):
                mx8 = t(wk, [P, 8], F32, "mx8")
                nc.vector.max(out=mx8[:], in_=cur[:])
                mi8 = t(wk, [P, 8], U32, "mi8")
                nc.vector.max_index(out=mi8[:], in_max=mx8[:], in_values=cur[:])
                nc.vector.tensor_copy(loc[:, r8 * 8 : (r8 + 1) * 8], mi8[:])
                if r8 < 7:
                    nc.vector.match_replace(out=nxt[:], in_to_replace=mx8[:], in_values=cur[:], imm_value=-1.0)
                    cur, nxt = nxt, cur
            rcnt = t(wk, [P, 1], F32, "rcnt")
            nc.vector.reduce_sum(rcnt[:], kept[:], axis=mybir.AxisListType.X)
            rkpp = psS.tile([P, 1], F32, tag="ss")
            nc.tensor.matmul(rkpp[:], lhsT=trl[:], rhs=rcnt[:], start=True, stop=True)
            nc.vector.tensor_copy(rkp[:], rkpp[:])
            nc.vector.tensor_add(rkpn[:], rkp[:], rcnt[:])

            for S in range(SC):
                rpS = t(wk, [P, 1], F32, "rpS")
                nc.vector.tensor_scalar_add(rpS[:], rkp[:], float(-P * S))
                rpnS = t(wk, [P, 1], F32, "rpnS")
                nc.vector.tensor_scalar_add(rpnS[:], rkpn[:], float(-P * S))
                selA = t(wk, [P, P], F32, "selA")
                nc.vector.tensor_scalar(out=selA[:], in0=isf[:], scalar1=rpS[:, :1], scalar2=None, op0=OP.is_ge)
                selB = t(wk, [P, P], F32, "selB")
                nc.vector.tensor_scalar(out=selB[:], in0=isf[:], scalar1=rpnS[:, :1], scalar2=None, op0=OP.is_lt)
                selO = t(wk, [P, P], F32, "selO")
                nc.vector.tensor_mul(selO[:], selA[:], selB[:])
                rap = psS.tile([1, P], F32, tag="ss")
                nc.tensor.matmul(rap[:], lhsT=rkp[:], rhs=selO[:], start=True, stop=True)
                psp = psS.tile([1, P], F32, tag="ss")
                nc.tensor.matmul(psp[:], lhsT=pvl[:], rhs=selO[:], start=True, stop=True)
                pst = t(wk, [1, P], F32, "pst")
                nc.vector.tensor_copy(pst[:], psp[:])
                rsr = t(wk, [1, P], F32, "rsr")
                nc.vector.tensor_scalar_add(rsr[:], irow[:], float(P * S))
                nc.vector.tensor_sub(rsr[:], rsr[:], rap[:])
                Tp = psS.tile([64, P], F32, tag="ss")
                nc.tensor.matmul(Tp[:], lhsT=loc[:], rhs=selO[:], start=True, stop=True)
                repp = psS.tile([64, P], F32, tag="ss")
                nc.tensor.matmul(repp[:], lhsT=o1x64[:], rhs=rsr[:], start=True, stop=True)
                Rm = t(wk, [64, P], F32, "Rm")
                nc.vector.tensor_tensor(out=Rm[:], in0=irp[:], in1=repp[:], op=OP.is_equal)
                RT = t(wk, [64, P], F32, "RT")
                nc.vector.tensor_mul(RT[:], Rm[:], Tp[:])
                srow = psS.tile([1, P], F32, tag="ss")
                nc.tensor.matmul(srow[:], lhsT=o64x1[:], rhs=RT[:], start=True, stop=True)
                idxr = t(wk, [1, P], F32, "idxr")
                nc.vector.tensor_scalar_mul(idxr[:], pst[:], float(JW))
                nc.vector.tensor_add(idxr[:], idxr[:], srow[:])
                itp = psS.tile([P, 1], F32, tag="ss")
                nc.tensor.transpose(out=itp[:], in_=idxr[:], identity=idf[:1, :1])
                nc.vector.tensor_copy(idxc[S][:], itp[:])

            for S in range(SC):
                xg = t(wk, [P, D], F32, "big4k", bufs=4)
                nc.gpsimd.indirect_dma_start(
                    out=xg[:], out_offset=None, in_=x.ap(),
                    in_offset=bass.IndirectOffsetOnAxis(ap=idxc[S][:, :1], axis=0))
                for k in range(8):
                    tp = psT.tile([P, P], F32, tag="tp")
                    nc.tensor.transpose(out=tp[:], in_=xg[:, k * P : (k + 1) * P], identity=idf[:])
                    nc.vector.tensor_copy(bufT[k][:, S * P : (S + 1) * P], tp[:])

        # --------- combine stats (precomputed, overlaps FFN schedule) ---------
        g1i = [t(pp, [P, 1], I32, f"g1i{c}") for c in range(8)]
        g2i = [t(pp, [P, 1], I32, f"g2i{c}") for c in range(8)]
        w1sv = [t(pp, [P, 1], F32, f"w1sv{c}") for c in range(8)]
        w2sv = [t(pp, [P, 1], F32, f"w2sv{c}") for c in range(8)]
        basep = psS.tile([1, E], F32, tag="ss")
        nc.tensor.matmul(basep[:], lhsT=s16[:], rhs=rpr[:], start=True, stop=True)
        cum = t(wk, [1, E], F32, "cum0")
        nc.vector.tensor_copy(cum[:], basep[:])
        for c in range(8):
            lpp = psS.tile([P, E], F32, tag="ss")
            nc.tensor.matmul(lpp[:], lhsT=o1x128[:], rhs=cum[:], start=True, stop=False)
            nc.tensor.matmul(lpp[:], lhsT=trl[:], rhs=lmask[c][:], start=False, stop=True)
            lp = t(wk, [P, E], F32, "lp")
            nc.vector.tensor_copy(lp[:], lpp[:])
            if c < 7:
                totp = psS.tile([1, E], F32, tag="ss")
                nc.tensor.matmul(totp[:], lhsT=o128x1[:], rhs=lmask[c][:], start=True, stop=True)
                ncum = t(wk, [1, E], F32, "cumN")
                nc.vector.tensor_add(ncum[:], cum[:], totp[:])
                cum = ncum
            va = t(wk, [P, E], F32, "va")
            nc.vector.tensor_scalar(out=va[:], in0=lp[:], scalar1=float(CAP) - 0.5, scalar2=None, op0=OP.is_le)
            lval = t(wk, [P, E], F32, "lval")
            nc.vector.tensor_mul(lval[:], va[:], lmask[c][:])
            lpc = t(wk, [P, E], F32, "lpc")
            nc.vector.tensor_scalar_min(lpc[:], lp[:], float(CAP - 1))
            # chunked outall layout: row = e*256 + (pos % 256) + 2048*(pos // 256)
            #   = e*256 + pos + 1792 * ch, ch = sum_k [pos >= 256k]
            chn = t(wk, [P, E], F32, "chn")
            cht = t(wk, [P, E], F32, "cht")
            nc.vector.tensor_scalar(out=chn[:], in0=lpc[:], scalar1=255.5, scalar2=None, op0=OP.is_ge)
            for kk in (512, 768, 1024):
                nc.vector.tensor_scalar(out=cht[:], in0=lpc[:], scalar1=float(kk) - 0.5, scalar2=None, op0=OP.is_ge)
                nc.vector.tensor_add(chn[:], chn[:], cht[:])
            rowid = t(wk, [P, E], F32, "rowid")
            nc.vector.tensor_scalar_mul(rowid[:], chn[:], 1792.0)
            nc.vector.tensor_add(rowid[:], rowid[:], lpc[:])
            nc.vector.tensor_add(rowid[:], rowid[:], ie256[:])
            gv = t(wk, [P, E], F32, "gv")
            nc.vector.tensor_mul(gv[:], gates[c][:], lval[:])
            is1 = t(wk, [P, E], F32, "is1")
            nc.vector.tensor_tensor(out=is1[:], in0=gates[c][:], in1=repl1[c][:], op=OP.is_gt)
            is2 = t(wk, [P, E], F32, "is2")
            nc.vector.tensor_tensor(out=is2[:], in0=repl1[c][:], in1=repl2[c][:], op=OP.is_gt)
            junk = t(wk, [P, E], F32, "junk")
            g1f = t(wk, [P, 1], F32, "g1f")
            nc.vector.tensor_mul(junk[:], is1[:], gv[:])
            nc.vector.reduce_sum(w1sv[c][:], junk[:], axis=mybir.AxisListType.X)
            nc.vector.tensor_mul(junk[:], is2[:], gv[:])
            nc.vector.reduce_sum(w2sv[c][:], junk[:], axis=mybir.AxisListType.X)
            nc.vector.tensor_mul(junk[:], is1[:], rowid[:])
            nc.vector.reduce_sum(g1f[:], junk[:], axis=mybir.AxisListType.X)
            nc.vector.tensor_copy(g1i[c][:], g1f[:])
            nc.vector.tensor_mul(junk[:], is2[:], rowid[:])
            nc.vector.reduce_sum(g1f[:], junk[:], axis=mybir.AxisListType.X)
            nc.vector.tensor_copy(g2i[c][:], g1f[:])

        # ---------------- Phase G: expert FFN ----------------
        with tc.tile_pool(name="fpool", bufs=1) as fp:
            # resident half of w2 (d 0:512); the other half streams per block
            w2r = [fp.tile([P, 512], BF16, tag=f"w2r{h}", name=f"w2r{h}") for h in range(32)]
            for h in range(32):
                nc.sync.dma_start(w2r[h][:], w2.ap()[h * P : (h + 1) * P, 0:512])
            hT = fp.tile([P, 32 * CB], BF16, tag="hT", name="hT")
            for cb in range(NCB):
                c0 = cb * CB
                for m in range(32):
                    ps1 = ps1p.tile([P, CB], F32, tag="ps1")
                    for k in range(8):
                        nc.tensor.matmul(
                            ps1[:], lhsT=w1sb[k][:, m * P : (m + 1) * P],
                            rhs=bufT[k][:, c0 : c0 + CB], start=(k == 0), stop=(k == 7))
                    nc.scalar.activation(
                        hT[:, m * CB : (m + 1) * CB], ps1[:], AF.Gelu, bias=b1t[:, m : m + 1])
                orow = t(wk, [P, D], BF16, "big2k", bufs=4)
                orow2 = t(wk, [P, D], BF16, "big2k", bufs=4)
                ot = (orow, orow2)
                w2s_tiles = []
                for h in range(32):
                    w2h = fp.tile([P, 512], BF16, tag="w2h", name="w2h", bufs=29)
                    nc.sync.dma_start(w2h[:], w2.ap()[h * P : (h + 1) * P, 512:1024])
                    w2s_tiles.append(w2h)
                for dn in range(2):
                    ps2a = ps2p.tile([P, 512], F32, tag="ps2a")
                    ps2b = ps2p.tile([P, 512], F32, tag="ps2b")
                    pt2 = (ps2a, ps2b)
                    for cc in range(2):
                        nc.tensor.matmul(
                            pt2[cc][:], lhsT=o1x128[:], rhs=b2_sb[:, dn * 512 : (dn + 1) * 512],
                            start=True, stop=False)
                    for h in range(32):
                        rhs = w2r[h] if dn == 0 else w2s_tiles[h]
                        for cc in range(2):
                            nc.tensor.matmul(
                                pt2[cc][:], lhsT=hT[:, h * CB + cc * P : h * CB + (cc + 1) * P],
                                rhs=rhs[:], start=False, stop=(h == 31))
                    for cc in range(2):
                        nc.vector.tensor_copy(ot[cc][:, dn * 512 : (dn + 1) * 512], pt2[cc][:])
                for cc in range(2):
                    nc.sync.dma_start(agin2.ap()[c0 + cc * P : c0 + (cc + 1) * P, :], ot[cc][:])
                nc.gpsimd.collective_compute(
                    "AllGather", OP.bypass, replica_groups=[list(range(NCORE))],
                    ins=[agin2.ap()[c0 : c0 + CB, :]],
                    outs=[outall.ap()[cb * (NCORE * CB) : (cb + 1) * (NCORE * CB), :]])

        # ---------------- Phase C: combine (gathers only) ----------------
        for c in range(8):
            r1 = t(wk, [P, D], BF16, "big2k", bufs=4)
            nc.gpsimd.indirect_dma_start(
                out=r1[:], out_offset=None, in_=outall.ap(),
                in_offset=bass.IndirectOffsetOnAxis(ap=g1i[c][:, :1], axis=0))
            r2 = t(wk, [P, D], BF16, "big2k", bufs=4)
            nc.gpsimd.indirect_dma_start(
                out=r2[:], out_offset=None, in_=outall.ap(),
                in_offset=bass.IndirectOffsetOnAxis(ap=g2i[c][:, :1], axis=0))
            y1 = t(wk, [P, D], F32, "big4k", bufs=4)
            nc.vector.tensor_scalar_mul(y1[:], r1[:], w1sv[c][:, :1])
            y2 = t(wk, [P, D], F32, "big4k", bufs=4)
            nc.scalar.activation(y2[:], r2[:], AF.Copy, scale=w2sv[c][:, :1])
            yc = t(wk, [P, D], F32, "big4k", bufs=4)
            nc.vector.tensor_add(yc[:], y1[:], y2[:])
            nc.sync.dma_start(y.ap()[c * P : (c + 1) * P, :], yc[:])

    nc.compile()
    return nc


def _make_in_maps(inputs):
    import ml_dtypes

    x = np.ascontiguousarray(np.asarray(inputs["x"], np.float32).reshape(NTOK, D))
    rw = np.ascontiguousarray(np.asarray(inputs["router_w"], np.float32))
    rb = np.ascontiguousarray(np.asarray(inputs["router_b"], np.float32)).reshape(1, E)
    w1 = np.asarray(inputs["w1"])
    w2 = np.asarray(inputs["w2"])
    b1 = np.asarray(inputs["b1"])
    b2 = np.asarray(inputs["b2"])
    in_maps = []
    for i in range(NCORE):
        oh = np.zeros((P, E), np.float32)
        oh[:, i] = 1.0
        s16 = np.zeros((P, 1), np.float32)
        s16[16 * i, 0] = 1.0
        in_maps.append({
            "x": x,
            "xs": np.ascontiguousarray(x[i * TSH : (i + 1) * TSH]),
            "rw": rw,
            "rb": rb,
            "w1": np.ascontiguousarray(np.asarray(w1[i], np.float32).astype(ml_dtypes.bfloat16)),
            "w2": np.ascontiguousarray(np.asarray(w2[i], np.float32).astype(ml_dtypes.bfloat16)),
            "b1": np.ascontiguousarray(np.asarray(b1[i], np.float32)).reshape(1, DH),
            "b2": np.ascontiguousarray(np.asarray(b2[i], np.float32)).reshape(1, D),
            "ohcol": oh,
            "sel16": s16,
        })
    return in_maps


def run(inputs, trace=False):
    if "nc" not in _CACHE:
        _CACHE["nc"] = _build()
    nc = _CACHE["nc"]
    in_maps = _make_in_maps(inputs)
    res = bass_utils.run_bass_kernel_spmd(
        nc, in_maps, core_ids=list(range(NCORE)), trace=trace
    )
    yfull = np.concatenate([res.results[i]["y"] for i in range(NCORE)], axis=0)
    return yfull.reshape(4, 2048, D), res


def kernel(**inputs) -> np.ndarray:
    y, _ = run(inputs, trace=False)
    return y
